# revision 1
# baseline (speedup 1.0000x reference)
"""Trainium2 Bass kernel for nn_Clar_Loss (NSML fusion-clarity MSE loss).

reference:
    x = (t+1)/2 ; s = sml(x) (8-neighbor abs-diff stencil, edge-replicate)
    nsml = G (*) s^2  (3x3 gaussian [[1,2,1],[2,4,2],[1,2,1]]/16, zero pad)
    loss = mean((nsml(A) - nsml(B))^2)

Algebra used here:
    sml((t+1)/2) = sml(t)/2          (translation invariant, pos. homogeneous)
    loss = sum((Graw (*) q)^2) / (N * 4096),  q = sA^2 - sB^2 (raw sml)
with Graw = [[1,2,1],[2,4,2],[1,2,1]] (integer), separable [1,2,1](x)[1,2,1].

Sharding: pure data-parallel over the batch dim (16 -> 2 per core); each core
returns a partial sum of (Graw(*)q)^2; host sums and rescales.

Layout: inputs are pre-padded on host to [H, W+2] (edge-replicated columns),
so every row-tile is ONE full [128, 1026] fp16 cast-DMA (partition p <->
image row off+p; off=0 / r0-2 / 896). All vertical/partition shifts are
folded into host-built 128x128 fp16 stationary matrices; image-boundary
clamping and the zero padding of s^2 are encoded as matrix-column edits, so
no partition ever needs pad data. The 4 |diff| fields are produced by a
custom fused DVE absdiff op, keeping each matmul's sync-wait count at 1.
"""

import os
from contextlib import ExitStack

import numpy as np

B, C, H, W = 16, 3, 1024, 1024
NCORES = 8
BPC = B // NCORES            # batch per core
NSITE = BPC * C              # image pairs per core
DIAG_W = 0.707
TILE_R = 124
NT = (H + TILE_R - 1) // TILE_R          # 9
FINAL_SCALE = 1.0 / (float(B * C * H * W) * 4096.0)

_CACHE = {}


def _tile_geom(t):
    """(r0, R, off): output rows [r0, r0+R), partition p <-> image row off+p."""
    r0 = t * TILE_R
    R = min(TILE_R, H - r0)
    if t == 0:
        off = 0
    elif R < TILE_R or r0 + 126 > H:
        off = H - 128
    else:
        off = r0 - 2
    return r0, R, off


# --------------------------------------------------------------------------
# host-built stationary matrices (lhsT layout [k, m]: out[m] += M[k,m]*in[k])
# --------------------------------------------------------------------------

def _matrices_for_tile(t):
    r0, R, off = _tile_geom(t)

    def sml_valid(m):
        r = off + m
        if not (max(r0 - 1, 0) <= r <= min(r0 + R, H - 1)):
            return False
        if m > 126 and r != H - 1:
            return False
        if m < 1 and r != 0:
            return False
        return True

    dv = np.zeros((128, 128), np.float32)    # dv[m] = x[m] - x[m-1]
    for m in range(1, 128):
        dv[m, m] = 1.0
        dv[m - 1, m] = -1.0

    av = np.zeros((128, 128), np.float32)    # a_v terms
    ihf = np.zeros((128, 128), np.float32)   # a_h at f
    ihf1 = np.zeros((128, 128), np.float32)  # a_h at f+1
    i71 = np.zeros((128, 128), np.float32)   # a_1 at f
    s71 = np.zeros((128, 128), np.float32)   # a_1[m+1] at f+1
    i72 = np.zeros((128, 128), np.float32)   # a_2 at f (img f)
    s72 = np.zeros((128, 128), np.float32)   # a_2[m+1] at f-1

    for m in range(128):
        if not sml_valid(m):
            continue
        r = off + m
        ihf[m, m] += 1.0           # |c-lf|
        ihf1[m, m] += 1.0          # |c-rt|
        if r >= 1:
            av[m, m] += 1.0        # |c-up| = a_v[m]
        if r <= H - 2:
            av[m + 1, m] += 1.0    # |c-dn| = a_v[m+1]
        if r == 0:                 # up-row clamps to own row
            ihf[m, m] += DIAG_W    # |c-ul| -> a_h[m, f]
            ihf1[m, m] += DIAG_W   # |c-ur| -> a_h[m, f+1]
        else:
            i71[m, m] += DIAG_W    # |c-ul| = a_1[m]
            i72[m, m] += DIAG_W    # |c-ur| = a_2[m]
        if r == H - 1:             # down-row clamps to own row
            ihf1[m, m] += DIAG_W   # |c-dr| -> a_h[m, f+1]
            ihf[m, m] += DIAG_W    # |c-dl| -> a_h[m, f]
        else:
            s71[m + 1, m] += DIAG_W  # |c-dr| = a_1[m+1] at f+1
            s72[m + 1, m] += DIAG_W  # |c-dl| = a_2[m+1] at f-1

    tri = np.zeros((128, 128), np.float32)
    for m in range(128):
        r = off + m
        if not (r0 <= r <= r0 + R - 1):
            continue
        for dr in (-1, 0, 1):
            k = m + dr
            if 0 <= k <= 127 and 0 <= off + k <= H - 1:
                tri[k, m] = 2.0 if dr == 0 else 1.0

    mats = {"DV": dv, "AV": av, "IHF": ihf, "IHF1": ihf1, "I71": i71,
            "S71": s71, "I72": i72, "S72": s72, "TRI": tri}
    return {k: v.astype(np.float16) for k, v in mats.items()}


def _build_weights():
    slots = {}
    packed = []
    index = {}
    for t in range(NT):
        for name, mat in _matrices_for_tile(t).items():
            key = mat.tobytes()
            if key not in slots:
                slots[key] = len(packed)
                packed.append(mat)
            index[(t, name)] = slots[key]
    ones = np.zeros((128, 128), np.float16)
    ones[:, 0] = 1.0
    index[("ones",)] = len(packed)
    packed.append(ones)
    wts = np.concatenate(packed, axis=1)  # [128, NW*128]
    return np.ascontiguousarray(wts), index


# --------------------------------------------------------------------------
# custom DVE op: absdiff  out = |in0 - in1|
# --------------------------------------------------------------------------

def _register_absdiff():
    from concourse import dve_ops
    from concourse.dve_spec import Spec, Src0, Src1, maxx, lower
    from concourse.dve_uop import DveOpSpec

    if any(op.name == "ABSDIFF_ANT" for op in dve_ops.OPS):
        return next(op for op in dve_ops.OPS if op.name == "ABSDIFF_ANT")

    spec = Spec(
        body=maxx(Src0 - Src1, Src1 - Src0),
        reference=lambda in0, in1, s0, s1, imm2: np.abs(
            in0.astype(np.float32) - in1.astype(np.float32)
        ),
    )
    opcode = max(dve_ops._SUB_OPCODE_FOR_NAME.values()) + 1
    assert opcode < 0x20
    shas = {}
    for ver in ("v3", "v4"):
        s = DveOpSpec(
            name="ABSDIFF_ANT",
            opcode=opcode,
            uops=lower(spec, ver=ver),
            rd1_en=True,
        )
        shas[ver] = s.sha(ver)
    op = dve_ops.DveOp("ABSDIFF_ANT", spec, subdim=False, uops_sha=shas)
    dve_ops.OPS.append(op)
    dve_ops._SUB_OPCODE_FOR_NAME["ABSDIFF_ANT"] = opcode
    dve_ops.CUSTOM_DVE_SPECS["ABSDIFF_ANT"] = spec
    return op


# --------------------------------------------------------------------------
# kernel build
# --------------------------------------------------------------------------

def _build(wts_np):
    import concourse.bass as bass
    import concourse.tile as tile
    from concourse import bacc, mybir

    F16 = mybir.dt.float16
    F32 = mybir.dt.float32
    AF = mybir.ActivationFunctionType
    OP = mybir.AluOpType

    absdiff = _register_absdiff()
    native_abs = bool(int(os.environ.get("CLAR_NATIVE_ABS", "1")))

    nc = bacc.Bacc()
    dA = nc.dram_tensor("TA", [NSITE, H, W + 2], F32, kind="ExternalInput")
    dB = nc.dram_tensor("TB", [NSITE, H, W + 2], F32, kind="ExternalInput")
    dW = nc.dram_tensor("WTS", list(wts_np.shape), F16, kind="ExternalInput")
    dO = nc.dram_tensor("OUT", [1, 1], F32, kind="ExternalOutput")

    with tile.TileContext(nc) as tc, ExitStack() as ctx:
        persist = ctx.enter_context(tc.tile_pool(name="persist", bufs=1))
        # bufs=4 with 4 DMAs/site-tile: slot reuse lands on the SAME
        # round-robin DMA lane, so the reload WAW is same-proc
        xp = ctx.enter_context(tc.tile_pool(name="xp", bufs=4))
        fields = ctx.enter_context(tc.tile_pool(name="fields", bufs=2))
        tails = ctx.enter_context(tc.tile_pool(name="tails", bufs=2))
        accs = ctx.enter_context(tc.tile_pool(name="accs", bufs=8))
        ps_sml = ctx.enter_context(tc.tile_pool(name="ps_sml", bufs=1, space="PSUM"))
        ps_dv = ctx.enter_context(tc.tile_pool(name="ps_dv", bufs=1, space="PSUM"))
        ps_r = ctx.enter_context(tc.tile_pool(name="ps_r", bufs=1, space="PSUM"))
        ps_misc = ctx.enter_context(tc.tile_pool(name="ps_misc", bufs=1, space="PSUM"))

        wsb = persist.tile(list(wts_np.shape), F16)
        nc.gpsimd.dma_start(wsb[:], dW[:])

        acc_slots = persist.tile([128, 64], F32)
        ones_f32 = persist.tile([128, 1], F32)

        oi = _WIDX[("ones",)]
        nc.vector.tensor_copy(ones_f32[:], wsb[:, oi * 128:oi * 128 + 1])

        # Sync-clock absorbers. Each 64B ISA instruction fits ~2 sync
        # commands (1 update + 1 wait), so every real op may carry at most
        # ONE cross-proc wait. Tiny engine ops ("carriers") pre-advance each
        # engine's observed clock of one other proc; emission order = sched
        # priority keeps them ahead of the real ops.
        dve_scr = persist.tile([1, 1], F32)
        act_scr = persist.tile([1, 1], F32)
        pool_scr = persist.tile([1, 2], F16)
        pe_scr = ps_misc.tile([1, 64], F32, tag="misc")

        def dve_sees(ap):
            nc.vector.tensor_copy(dve_scr[:], ap[0:1, 0:1])

        def act_sees(ap):
            nc.scalar.copy(act_scr[:], ap[0:1, 0:1])

        def pe_sees(ap):
            nc.tensor.matmul(pe_scr[0:1, 0:1], ap[:, 0:1], ap[:, 0:1],
                             start=True, stop=True)

        # pre-initialize the xu pool slots so partition 0 (never DMA'd) is
        # finite data, not virgin SBUF
        for _ in range(4):
            xu0 = xp.tile([128, 1026], F16, tag="xu")
            nc.vector.memset(xu0[0:1, :], 0.0)
        for _ in range(2):
            for tg in ("a_h", "a_1", "a_2"):
                f0 = fields.tile([128, 1026], F16, tag=tg, name=tg)
                nc.vector.memset(f0[0:1, 1024:1026], 0.0)

        def WT(t, name):
            i = _WIDX[(t, name)]
            return wsb[:, i * 128:(i + 1) * 128]


        def image_pipeline(dram, s, t, off, tag):
            x = xp.tile([128, 1026], F16, tag="x")
            nc.gpsimd.dma_start(x[:, :], dram[s, off:off + 128, :])
            # xu[p] = image row off+p-1 (partition-shifted SBUF copy)
            xu = xp.tile([128, 1026], F16, tag="xu")
            nc.sync.dma_start(xu[1:128, :], x[0:127, :])

            # |diff| fields (fused sub+abs custom DVE op, or native fallback)
            # col conventions (img col of sb col j):
            #   a_v: j-1   a_h: j    a_1: j    a_2: j-1
            # a_v on PE (bidiagonal matrix) + ACT abs; PSUM-relieving the DVE
            dv_ps = ps_dv.tile([128, 1024], F32, tag="dv")
            for c in range(2):
                nc.tensor.matmul(
                    dv_ps[:, c * 512:(c + 1) * 512], WT(t, "DV"),
                    x[:, 1 + c * 512: 513 + c * 512], start=True, stop=True)
            a_v = fields.tile([128, 1024], F16, tag="a_v")
            nc.scalar.activation(a_v[:], dv_ps[:], AF.Abs)

            def absfield(tag_, i0, i1):
                # sub on the 1025 valid cols (odd shift forces 1x mode);
                # abs full 1026-wide so it runs in the 2x packed mode
                f = fields.tile([128, 1026], F16, tag=tag_, name=tag_)
                nc.vector.tensor_sub(f[:, 0:1025], i0, i1)
                if tag_ in ("a_1", "a_2"):
                    # balance: diag-field |.| runs on ScalarE, a_h stays DVE
                    nc.scalar.activation(f[:, 0:1025], f[:, 0:1025], AF.Abs)
                else:
                    nc.vector.scalar_tensor_tensor(
                        f[:, :], f[:, :], -1.0, f[:, :], OP.mult, OP.max)
                return f
            a_h = absfield("a_h", x[:, 1:1026], x[:, 0:1025])
            a_1 = absfield("a_1", x[:, 1:1026], xu[:, 0:1025])
            a_2 = absfield("a_2", x[:, 0:1025], xu[:, 1:1026])


            # sml assembly on PE (PSUM accumulate), 2 chunks of 512 cols
            sml = ps_sml.tile([128, 1024], F32, tag="sml")
            for c in range(2):
                F0 = c * 512
                o = sml[:, F0:F0 + 512]
                mm = nc.tensor.matmul
                mm(o, WT(t, "AV"), a_v[:, F0:F0 + 512], start=True, stop=False)
                mm(o, WT(t, "IHF"), a_h[:, F0:F0 + 512], start=False, stop=False)
                mm(o, WT(t, "IHF1"), a_h[:, F0 + 1:F0 + 513], start=False, stop=False)
                mm(o, WT(t, "I71"), a_1[:, F0:F0 + 512], start=False, stop=False)
                mm(o, WT(t, "S71"), a_1[:, F0 + 1:F0 + 513], start=False, stop=False)
                mm(o, WT(t, "I72"), a_2[:, F0 + 1:F0 + 513], start=False, stop=False)
                mm(o, WT(t, "S72"), a_2[:, F0:F0 + 512], start=False, stop=True)

            s2 = fields.tile([128, 1024], F16, tag=f"s2{tag}")
            nc.scalar.activation(s2[:], sml[:], AF.Square)
            return s2

        for s in range(NSITE):
            for t in range(NT):
                r0, R, off = _tile_geom(t)
                s2a = image_pipeline(dA, s, t, off, "A")
                s2b = image_pipeline(dB, s, t, off, "B")

                q = tails.tile([128, 1026], F16, tag="q")
                nc.vector.memset(q[:, 0:1], 0.0)
                nc.vector.memset(q[:, 1025:1026], 0.0)
                nc.vector.scalar_tensor_tensor(
                    q[:, 1:1025], s2b[:], -1.0, s2a[:], OP.mult, OP.add)

                tt = tails.tile([128, 1024], F16, tag="tt")
                nc.vector.tensor_add(tt[:], q[:, 0:1024], q[:, 2:1026])
                th = tails.tile([128, 1024], F16, tag="th")
                nc.vector.scalar_tensor_tensor(
                    th[:], q[:, 1:1025], 2.0, tt[:], OP.mult, OP.add)

                r = ps_r.tile([128, 1024], F32, tag="r")
                for c in range(2):
                    nc.tensor.matmul(
                        r[:, c * 512:(c + 1) * 512], WT(t, "TRI"),
                        th[:, c * 512:(c + 1) * 512], start=True, stop=True,
                    )

                junk = tails.tile([128, 1024], F32, tag="junk")
                idx = s * NT + t
                nc.scalar.activation(junk[:], r[:], AF.Square,
                                     accum_out=acc_slots[:, idx:idx + 1])

        tot_ps = ps_misc.tile([1, 64], F32, tag="misc2")
        nc.tensor.matmul(tot_ps[:], ones_f32[:], acc_slots[:, 0:64],
                         start=True, stop=True)
        out_sb = persist.tile([1, 1], F32)
        nc.vector.reduce_sum(out_sb[:], tot_ps[:], axis=mybir.AxisListType.X)
        nc.sync.dma_start(dO[:], out_sb[:])

    nc.compile()
    return nc


_WIDX = None


def _get_module():
    global _WIDX
    if "nc" in _CACHE:
        return _CACHE["nc"], _CACHE["wts"]
    wts_np, widx = _build_weights()
    _WIDX = widx
    nc = _build(wts_np)
    _CACHE["nc"] = nc
    _CACHE["wts"] = wts_np
    return nc, wts_np


def _pad_cols(a):
    # [NSITE, H, W] -> [NSITE, H, W+2] with edge-replicated columns
    return np.ascontiguousarray(
        np.concatenate([a[:, :, :1], a, a[:, :, -1:]], axis=2))


def kernel(TensorA, TensorB):
    from concourse.bass_utils import run_bass_kernel_spmd

    nc, wts_np = _get_module()
    A = np.asarray(TensorA, dtype=np.float32).reshape(B * C, H, W)
    Bv = np.asarray(TensorB, dtype=np.float32).reshape(B * C, H, W)
    in_maps = []
    for c in range(NCORES):
        in_maps.append({
            "TA": _pad_cols(A[c * NSITE:(c + 1) * NSITE]),
            "TB": _pad_cols(Bv[c * NSITE:(c + 1) * NSITE]),
            "WTS": wts_np,
        })
    res = run_bass_kernel_spmd(
        nc, in_maps, core_ids=list(range(NCORES)),
        trace=bool(int(os.environ.get("CLAR_TRACE", "0"))),
    )
    _CACHE["last_results"] = res
    total = sum(float(r["OUT"][0, 0]) for r in res.results)
    return np.float32(total * FINAL_SCALE)



# revision 18
# speedup vs baseline: 1.4363x; 1.4363x over previous
"""Trainium2 Bass kernel for nn_Clar_Loss (NSML fusion-clarity MSE loss).

reference:
    x = (t+1)/2 ; s = sml(x) (8-neighbor abs-diff stencil, edge-replicate)
    nsml = G (*) s^2  (3x3 gaussian [[1,2,1],[2,4,2],[1,2,1]]/16, zero pad)
    loss = mean((nsml(A) - nsml(B))^2)

Algebra used here:
    sml((t+1)/2) = sml(t)/2          (translation invariant, pos. homogeneous)
    loss = sum((Graw (*) q)^2) / (N * 4096),  q = sA^2 - sB^2 (raw sml)
with Graw = [[1,2,1],[2,4,2],[1,2,1]] (integer), separable [1,2,1](x)[1,2,1].

Sharding: pure data-parallel over the batch dim (16 -> 2 per core); each core
returns a partial sum of (Graw(*)q)^2; host sums and rescales.

Layout: inputs are pre-padded on host to [H, W+2] (edge-replicated columns)
and pre-cast to fp16, so every row-tile is ONE full [128, 1026] fp16 DMA
issued from the SP queue (no gpsimd cast-DMA needed). Partition p <-> image
row off+p; off=0 / r0-2 / 896. Vertical/partition shifts are folded into
host-built 128x128 fp16 stationary matrices; image-boundary clamping and the
zero padding of s^2 are encoded as matrix-column edits, so no partition ever
needs pad data.

All four |diff| fields are produced by a custom fused DVE absdiff op
(max(a-b, b-a)) registered with a hand-written 2X_1PORT uop program and
perf_max=1, so fp16 packed operands run at 2 elem/lane/cycle. The horizontal
gaussian tail runs q/t1 on DVE and th on the Pool engine (gpsimd STT);
vertical gaussian + field assembly are PE matmuls; squares + the global
accumulation run on ACT.
"""

import os
from contextlib import ExitStack

import numpy as np

B, C, H, W = 16, 3, 1024, 1024
NCORES = 8
BPC = B // NCORES            # batch per core
NSITE = BPC * C              # image pairs per core
DIAG_W = 0.707
TILE_R = 124
NFULL = 8                    # full tiles 0..7 cover output rows 0..991
STUB_OFF = 990               # stub block: x rows 990..1023
STUB_R0 = 992                # stub outputs rows 992..1023 (32 rows)
STUB_NROWS = 34
STUB_SITES = 3               # sites packed per stub tile (3*34 = 102 <= 128)
FINAL_SCALE = 1.0 / (float(B * C * H * W) * 4096.0)

_CACHE = {}


def _tile_geom(t):
    """(r0, R, off): output rows [r0, r0+R), partition p <-> image row off+p."""
    r0 = t * TILE_R
    R = min(TILE_R, H - r0)
    if t == 0:
        off = 0
    elif R < TILE_R or r0 + 126 > H:
        off = H - 128
    else:
        off = r0 - 2
    return r0, R, off


# --------------------------------------------------------------------------
# host-built stationary matrices (lhsT layout [k, m]: out[m] += M[k,m]*in[k])
# --------------------------------------------------------------------------

def _matrices_for_tile(t):
    r0, R, off = _tile_geom(t)

    def sml_valid(m):
        r = off + m
        if not (max(r0 - 1, 0) <= r <= min(r0 + R, H - 1)):
            return False
        if m > 126 and r != H - 1:
            return False
        if m < 1 and r != 0:
            return False
        return True

    av = np.zeros((128, 128), np.float32)    # a_v terms
    ihf = np.zeros((128, 128), np.float32)   # a_h at f
    ihf1 = np.zeros((128, 128), np.float32)  # a_h at f+1
    i71 = np.zeros((128, 128), np.float32)   # a_1 at f
    s71 = np.zeros((128, 128), np.float32)   # a_1[m+1] at f+1
    i72 = np.zeros((128, 128), np.float32)   # a_2 at f (img f)
    s72 = np.zeros((128, 128), np.float32)   # a_2[m+1] at f-1

    for m in range(128):
        if not sml_valid(m):
            continue
        r = off + m
        ihf[m, m] += 1.0           # |c-lf|
        ihf1[m, m] += 1.0          # |c-rt|
        if r >= 1:
            av[m, m] += 1.0        # |c-up| = a_v[m]
        if r <= H - 2:
            av[m + 1, m] += 1.0    # |c-dn| = a_v[m+1]
        if r == 0:                 # up-row clamps to own row
            ihf[m, m] += DIAG_W    # |c-ul| -> a_h[m, f]
            ihf1[m, m] += DIAG_W   # |c-ur| -> a_h[m, f+1]
        else:
            i71[m, m] += DIAG_W    # |c-ul| = a_1[m]
            i72[m, m] += DIAG_W    # |c-ur| = a_2[m]
        if r == H - 1:             # down-row clamps to own row
            ihf1[m, m] += DIAG_W   # |c-dr| -> a_h[m, f+1]
            ihf[m, m] += DIAG_W    # |c-dl| -> a_h[m, f]
        else:
            s71[m + 1, m] += DIAG_W  # |c-dr| = a_1[m+1] at f+1
            s72[m + 1, m] += DIAG_W  # |c-dl| = a_2[m+1] at f-1

    tri = np.zeros((128, 128), np.float32)
    for m in range(128):
        r = off + m
        if not (r0 <= r <= r0 + R - 1):
            continue
        for dr in (-1, 0, 1):
            k = m + dr
            if 0 <= k <= 127 and 0 <= off + k <= H - 1:
                tri[k, m] = 2.0 if dr == 0 else 1.0

    mats = {"AV": av, "IHF": ihf, "IHF1": ihf1, "I71": i71,
            "S71": s71, "I72": i72, "S72": s72, "TRI": tri}
    return {k: v.astype(np.float16) for k, v in mats.items()}


def _build_weights():
    slots = {}
    packed = []
    index = {}
    for t in range(NT):
        for name, mat in _matrices_for_tile(t).items():
            key = mat.tobytes()
            if key not in slots:
                slots[key] = len(packed)
                packed.append(mat)
            index[(t, name)] = slots[key]
    ones = np.zeros((128, 128), np.float16)
    ones[:, 0] = 1.0
    index[("ones",)] = len(packed)
    packed.append(ones)
    wts = np.concatenate(packed, axis=1)  # [128, NW*128]
    return np.ascontiguousarray(wts), index


# --------------------------------------------------------------------------
# custom DVE op: absdiff  out = |in0 - in1|, with a 2X_1PORT uop program
# --------------------------------------------------------------------------

def _register_absdiff():
    from concourse import dve_ops
    from concourse.dve_spec import Spec, Src0, Src1, maxx, lower
    from concourse.dve_uop import (
        DveOpSpec, UopConfig, UopDpConfig, AluOp, AluInp, DelayInp,
        InpSel, OutPath, OutSel,
    )
    from concourse.dve_ops import _COMPILE_CACHE

    NAME = "ABSDIFF_ANT"
    for op in dve_ops.OPS:
        if op.name == NAME:
            return op

    spec = Spec(
        body=maxx(Src0 - Src1, Src1 - Src0),
        reference=lambda in0, in1, s0, s1, imm2: np.abs(
            in0.astype(np.float32) - in1.astype(np.float32)
        ),
    )

    def build_2x(u1x):
        import copy

        u = copy.deepcopy(u1x)
        u.inp[3] = InpSel.SRC_0_HI
        u.inp[4] = InpSel.SRC_1_HI
        u.inp_enable[3] = 1
        u.inp_enable[4] = 1

        P = DelayInp.PREV_DELAY
        A = DelayInp.PREV_ALU_OUT

        def blk(op_, a, b, delay_sel, delay_en):
            d = UopDpConfig()
            d.op = op_
            d.alu_src0 = a
            d.alu_src1 = b
            d.alu_out_enable = 1
            d.delay = list(delay_sel) + [A] * (len(d.delay) - len(delay_sel))
            d.delay_enable = list(delay_en) + [0] * (
                len(d.delay_enable) - len(delay_en)
            )
            return d

        D0, D1, D2, D3 = (
            AluInp.PREV_DELAY_0, AluInp.PREV_DELAY_1,
            AluInp.PREV_DELAY_2, AluInp.PREV_DELAY_3,
        )
        ALU = AluInp.PREV_ALU_OUT
        u.datapath_config = [
            # lo: |s0-s1| on blks 0-2; hi (SRC_*_HI) on blks 3-5; the lo
            # result rides delay lane 0 to the output stage.
            blk(AluOp.SUBTRACT, D0, D1, [P, P, P, P], [1, 1, 1, 1]),
            blk(AluOp.SUBTRACT, D1, D0, [A, P, P, P], [1, 0, 1, 1]),
            blk(AluOp.MAX, D0, ALU, [P, P, P, P], [1, 0, 1, 1]),
            blk(AluOp.SUBTRACT, D2, D3, [A, P, P, P], [1, 0, 1, 1]),
            blk(AluOp.SUBTRACT, D3, D2, [P, A, P, P], [1, 1, 0, 0]),
            blk(AluOp.MAX, D1, ALU, [P, P, P, P], [1, 0, 0, 0]),
            blk(AluOp.BYPASS, ALU, ALU, [P, P, P, P], [1, 0, 0, 0]),
            blk(AluOp.BYPASS, ALU, ALU, [P, P, P, P], [1, 0, 0, 0]),
        ]
        u.out = dict(u.out)
        u.out[OutPath.WR0_LO] = OutSel.DELAY_0
        u.out[OutPath.WR0_HI] = OutSel.ALU_OUT
        u.out_enable = dict(u.out_enable)
        u.out_enable[OutPath.WR0_LO] = 1
        u.out_enable[OutPath.WR0_HI] = 1
        return u

    opcode = max(dve_ops._SUB_OPCODE_FOR_NAME.values()) + 1
    assert opcode < 0x20
    shas = {}
    specs_by_ver = {}
    for ver in ("v3", "v4"):
        uops = lower(spec, ver=ver)
        assert len(uops) == 1
        u2x = build_2x(uops[0])
        u2x.validate(ver)
        s = DveOpSpec(
            name=NAME, opcode=opcode, uops=uops, uops_2x=[u2x],
            perf_max=1, rd1_en=True,
        )
        shas[ver] = s.sha(ver)
        specs_by_ver[ver] = s

    op = dve_ops.DveOp(NAME, spec, subdim=False, uops_sha=shas)
    dve_ops.OPS.append(op)
    dve_ops._SUB_OPCODE_FOR_NAME[NAME] = opcode
    dve_ops.CUSTOM_DVE_SPECS[NAME] = spec
    # Seed the compile cache so both instruction emission and
    # dve_table_for_ops use THIS spec (with the 2x program).
    for ver, s in specs_by_ver.items():
        s.opcode = dve_ops.get_dve_sub_opcode(NAME)
        _COMPILE_CACHE[(NAME, ver)] = s
    return op


# --------------------------------------------------------------------------
# kernel build
# --------------------------------------------------------------------------

def _build(wts_np):
    import concourse.bass as bass
    import concourse.tile as tile
    from concourse import bacc, mybir

    F16 = mybir.dt.float16
    F32 = mybir.dt.float32
    AF = mybir.ActivationFunctionType
    OP = mybir.AluOpType

    absdiff = _register_absdiff()

    nc = bacc.Bacc()
    dA = nc.dram_tensor("TA", [NSITE, H, W + 2], F16, kind="ExternalInput")
    dB = nc.dram_tensor("TB", [NSITE, H, W + 2], F16, kind="ExternalInput")
    dW = nc.dram_tensor("WTS", list(wts_np.shape), F16, kind="ExternalInput")
    dO = nc.dram_tensor("OUT", [1, 1], F32, kind="ExternalOutput")

    custom_insts = []

    def absd(out_ap, in0_ap, in1_ap):
        bi = nc.vector._custom_dve(absdiff, out=out_ap, in0=in0_ap, in1=in1_ap)
        custom_insts.append(bi)
        return bi

    with tile.TileContext(nc) as tc, ExitStack() as ctx:
        persist = ctx.enter_context(tc.tile_pool(name="persist", bufs=1))
        xp = ctx.enter_context(tc.tile_pool(name="xp", bufs=6))
        fields = ctx.enter_context(tc.tile_pool(name="fields", bufs=2))
        tails = ctx.enter_context(tc.tile_pool(name="tails", bufs=3))
        ps_sml = ctx.enter_context(tc.tile_pool(name="ps_sml", bufs=1, space="PSUM"))
        ps_r = ctx.enter_context(tc.tile_pool(name="ps_r", bufs=2, space="PSUM"))

        wsb = persist.tile(list(wts_np.shape), F16)
        nc.sync.dma_start(wsb[:], dW[:])

        acc_slots = persist.tile([128, 64], F32)
        ones_f32 = persist.tile([128, 1], F32)

        oi = _WIDX[("ones",)]
        nc.vector.tensor_copy(ones_f32[:], wsb[:, oi * 128:oi * 128 + 1])

        # pre-initialize pool slots so partitions/cols never touched by DMA
        # or compute hold finite data, not virgin SBUF
        for _ in range(4):
            xu0 = xp.tile([128, 1026], F16, tag="xu")
            nc.vector.memset(xu0[0:1, :], 0.0)
        for _ in range(2):
            for tg in ("a_h", "a_1", "a_2"):
                f0 = fields.tile([128, 1026], F16, tag=tg, name=tg)
                nc.vector.memset(f0[:, 1024:1026], 0.0)

        def WT(t, name):
            i = _WIDX[(t, name)]
            return wsb[:, i * 128:(i + 1) * 128]

        def image_pipeline(dram, s, t, off, tag):
            x = xp.tile([128, 1026], F16, tag="x")
            nc.sync.dma_start(x[:, :], dram[s, off:off + 128, :])
            # xu[p] = image row off+p-1 (partition-shifted SBUF copy).
            # Issued from the scalar queue so the x->xu data dependency
            # doesn't stall the sync queue's x prefetch stream.
            xu = xp.tile([128, 1026], F16, tag="xu")
            nc.scalar.dma_start(xu[1:128, :], x[0:127, :])

            # |diff| fields via the fused absdiff op
            # col conventions (img col of sb col j):
            #   a_v: j   a_h: j    a_1: j    a_2: j-1
            a_v = fields.tile([128, 1024], F16, tag="a_v")
            absd(a_v[:, :], x[:, 1:1025], xu[:, 1:1025])
            a_h = fields.tile([128, 1026], F16, tag="a_h", name="a_h")
            absd(a_h[:, 0:1025], x[:, 1:1026], x[:, 0:1025])
            a_1 = fields.tile([128, 1026], F16, tag="a_1", name="a_1")
            absd(a_1[:, 0:1025], x[:, 1:1026], xu[:, 0:1025])
            a_2 = fields.tile([128, 1026], F16, tag="a_2", name="a_2")
            absd(a_2[:, 0:1025], x[:, 0:1025], xu[:, 1:1026])

            # sml assembly on PE (PSUM accumulate), 2 chunks of 512 cols
            sml = ps_sml.tile([128, 1024], F32, tag=f"sml{tag}")
            for c in range(2):
                F0 = c * 512
                o = sml[:, F0:F0 + 512]
                mm = nc.tensor.matmul
                mm(o, WT(t, "AV"), a_v[:, F0:F0 + 512], start=True, stop=False)
                mm(o, WT(t, "IHF"), a_h[:, F0:F0 + 512], start=False, stop=False)
                mm(o, WT(t, "IHF1"), a_h[:, F0 + 1:F0 + 513], start=False, stop=False)
                mm(o, WT(t, "I71"), a_1[:, F0:F0 + 512], start=False, stop=False)
                mm(o, WT(t, "S71"), a_1[:, F0 + 1:F0 + 513], start=False, stop=False)
                mm(o, WT(t, "I72"), a_2[:, F0 + 1:F0 + 513], start=False, stop=False)
                mm(o, WT(t, "S72"), a_2[:, F0:F0 + 512], start=False, stop=True)

            s2 = fields.tile([128, 1024], F16, tag=f"s2{tag}")
            nc.scalar.activation(s2[:], sml[:], AF.Square)
            return s2

        # --- tail stages, software-pipelined across iterations -----------
        def emit_q(p):
            # q = s2a - s2b with zeroed edge cols (Pool; DVE is tighter)
            q = tails.tile([128, 1026], F16, tag="q")
            nc.gpsimd.memset(q[:, 0:1], 0.0)
            nc.gpsimd.memset(q[:, 1025:1026], 0.0)
            nc.gpsimd.tensor_sub(q[:, 1:1025], p["s2a"][:], p["s2b"][:])
            p["q"] = q

        def emit_t1th(p):
            # horizontal [1,2,1] as two window-2 adds on DVE:
            # u[j] = q[j] + q[j+1]; th[c] = u[c] + u[c+1]
            q = p["q"]
            u = tails.tile([128, 1026], F16, tag="t1")
            nc.vector.tensor_add(u[:, 0:1025], q[:, 0:1025], q[:, 1:1026])
            th = tails.tile([128, 1024], F16, tag="th")
            nc.vector.tensor_add(th[:], u[:, 0:1024], u[:, 1:1025])
            p["th"] = th

        def emit_trijunk(p):
            th, t, idx = p["th"], p["t"], p["idx"]
            r = ps_r.tile([128, 1024], F32, tag="r")
            for c in range(2):
                nc.tensor.matmul(
                    r[:, c * 512:(c + 1) * 512], WT(t, "TRI"),
                    th[:, c * 512:(c + 1) * 512], start=True, stop=True,
                )
            junk = tails.tile([128, 1024], F32, tag="junk")
            nc.scalar.activation(junk[:], r[:], AF.Square,
                                 accum_out=acc_slots[:, idx:idx + 1])

        # 3-stage pipeline: pipes(k) | q(k-1) | t1/th(k-1) | TRI/junk(k-2)
        # so PE's TRI and Pool's th always have a full iteration of slack.
        items = []
        units = [(s, t) for s in range(NSITE) for t in range(NT)]
        for k, (s, t) in enumerate(units):
            r0, R, off = _tile_geom(t)
            s2a = image_pipeline(dA, s, t, off, "A")
            s2b = image_pipeline(dB, s, t, off, "B")
            items.append({"s2a": s2a, "s2b": s2b, "t": t, "idx": s * NT + t})
            if k >= 1:
                emit_q(items[k - 1])
                emit_t1th(items[k - 1])
            if k >= 2:
                emit_trijunk(items[k - 2])
        last = len(units) - 1
        emit_q(items[last])
        emit_t1th(items[last])
        emit_trijunk(items[last - 1])
        emit_trijunk(items[last])

        tot_full = ps_r.tile([128, 1024], F32, tag="r")
        tot_ps = tot_full[0:1, 0:64]
        nc.tensor.matmul(tot_ps, ones_f32[:], acc_slots[:, 0:64],
                         start=True, stop=True)
        out_sb = persist.tile([1, 1], F32)
        nc.vector.reduce_sum(out_sb[:], tot_ps, axis=mybir.AxisListType.X)
        nc.sync.dma_start(dO[:], out_sb[:])

    for bi in custom_insts:
        bi.ins.perf_max = 1
    nc.compile()
    return nc


_WIDX = None


def _get_module():
    global _WIDX
    if "nc" in _CACHE:
        return _CACHE["nc"], _CACHE["wts"]
    wts_np, widx = _build_weights()
    _WIDX = widx
    nc = _build(wts_np)
    _CACHE["nc"] = nc
    _CACHE["wts"] = wts_np
    return nc, wts_np


def _pad_cols(a):
    # [NSITE, H, W] -> [NSITE, H, W+2] fp16 with edge-replicated columns
    out = np.empty((a.shape[0], a.shape[1], a.shape[2] + 2), np.float16)
    out[:, :, 1:-1] = a
    out[:, :, 0] = a[:, :, 0]
    out[:, :, -1] = a[:, :, -1]
    return out


def kernel(TensorA, TensorB):
    from concourse.bass_utils import run_bass_kernel_spmd

    nc, wts_np = _get_module()
    A = np.asarray(TensorA, dtype=np.float32).reshape(B * C, H, W)
    Bv = np.asarray(TensorB, dtype=np.float32).reshape(B * C, H, W)
    in_maps = []
    for c in range(NCORES):
        in_maps.append({
            "TA": _pad_cols(A[c * NSITE:(c + 1) * NSITE]),
            "TB": _pad_cols(Bv[c * NSITE:(c + 1) * NSITE]),
            "WTS": wts_np,
        })
    res = run_bass_kernel_spmd(
        nc, in_maps, core_ids=list(range(NCORES)),
        trace=bool(int(os.environ.get("CLAR_TRACE", "0"))),
    )
    _CACHE["last_results"] = res
    total = sum(float(r["OUT"][0, 0]) for r in res.results)
    return np.float32(total * FINAL_SCALE)


# revision 24
# speedup vs baseline: 1.5407x; 1.0727x over previous
"""Trainium2 Bass kernel for nn_Clar_Loss (NSML fusion-clarity MSE loss).

reference:
    x = (t+1)/2 ; s = sml(x) (8-neighbor abs-diff stencil, edge-replicate)
    nsml = G (*) s^2  (3x3 gaussian [[1,2,1],[2,4,2],[1,2,1]]/16, zero pad)
    loss = mean((nsml(A) - nsml(B))^2)

Algebra used here:
    sml((t+1)/2) = sml(t)/2          (translation invariant, pos. homogeneous)
    loss = sum((Graw (*) q)^2) / (N * 4096),  q = sA^2 - sB^2 (raw sml)
with Graw = [[1,2,1],[2,4,2],[1,2,1]] (integer), separable [1,2,1](x)[1,2,1].

Sharding: pure data-parallel over the batch dim (16 -> 2 per core); each core
returns a partial sum of (Graw(*)q)^2; host sums and rescales.

Layout: inputs are pre-padded on host to [H, W+2] (edge-replicated columns)
and pre-cast to fp16, so every row-tile is ONE full [128, 1026] fp16 DMA
issued from the SP queue (no gpsimd cast-DMA needed). Partition p <-> image
row off+p; off=0 / r0-2 / 896. Vertical/partition shifts are folded into
host-built 128x128 fp16 stationary matrices; image-boundary clamping and the
zero padding of s^2 are encoded as matrix-column edits, so no partition ever
needs pad data.

All four |diff| fields are produced by a custom fused DVE absdiff op
(max(a-b, b-a)) registered with a hand-written 2X_1PORT uop program and
perf_max=1, so fp16 packed operands run at 2 elem/lane/cycle. The horizontal
gaussian tail runs q/t1 on DVE and th on the Pool engine (gpsimd STT);
vertical gaussian + field assembly are PE matmuls; squares + the global
accumulation run on ACT.
"""

import os
from contextlib import ExitStack

import numpy as np

B, C, H, W = 16, 3, 1024, 1024
NCORES = 8
BPC = B // NCORES            # batch per core
NSITE = BPC * C              # image pairs per core
DIAG_W = 0.707
TILE_R = 124
NFULL = 8                    # full tiles 0..7 cover output rows 0..991
STUB_OFF = 990               # stub block: x rows 990..1023
STUB_R0 = 992                # stub outputs rows 992..1023 (32 rows)
STUB_NROWS = 34
STUB_SITES = 3               # sites packed per stub tile (3*34 = 102 <= 128)
FINAL_SCALE = 1.0 / (float(B * C * H * W) * 4096.0)

_CACHE = {}


def _tile_geom(t):
    """(r0, R, off): output rows [r0, r0+R), partition p <-> image row off+p."""
    r0 = t * TILE_R
    R = TILE_R
    off = 0 if t == 0 else r0 - 2
    return r0, R, off


# --------------------------------------------------------------------------
# host-built stationary matrices (lhsT layout [k, m]: out[m] += M[k,m]*in[k])
# --------------------------------------------------------------------------

def _add_block(mats, pbase, off, r0, R, nrows):
    """Emit one vertical block's stencil/tri coefficients at partition base
    ``pbase``: block partitions m=0..nrows-1 <-> image rows off+m."""
    av, ihf, ihf1 = mats["AV"], mats["IHF"], mats["IHF1"]
    i71, s71, i72, s72, tri = (
        mats["I71"], mats["S71"], mats["I72"], mats["S72"], mats["TRI"])

    def sml_valid(m):
        r = off + m
        if not (max(r0 - 1, 0) <= r <= min(r0 + R, H - 1)):
            return False
        if m > nrows - 2 and r != H - 1:
            return False
        if m < 1 and r != 0:
            return False
        return True

    for m in range(nrows):
        if not sml_valid(m):
            continue
        r = off + m
        P = pbase + m
        ihf[P, P] += 1.0           # |c-lf|
        ihf1[P, P] += 1.0          # |c-rt|
        if r >= 1:
            av[P, P] += 1.0        # |c-up| = a_v[m]
        if r <= H - 2:
            av[P + 1, P] += 1.0    # |c-dn| = a_v[m+1]
        if r == 0:                 # up-row clamps to own row
            ihf[P, P] += DIAG_W    # |c-ul| -> a_h[m, f]
            ihf1[P, P] += DIAG_W   # |c-ur| -> a_h[m, f+1]
        else:
            i71[P, P] += DIAG_W    # |c-ul| = a_1[m]
            i72[P, P] += DIAG_W    # |c-ur| = a_2[m]
        if r == H - 1:             # down-row clamps to own row
            ihf1[P, P] += DIAG_W   # |c-dr| -> a_h[m, f+1]
            ihf[P, P] += DIAG_W    # |c-dl| -> a_h[m, f]
        else:
            s71[P + 1, P] += DIAG_W  # |c-dr| = a_1[m+1] at f+1
            s72[P + 1, P] += DIAG_W  # |c-dl| = a_2[m+1] at f-1

    for m in range(nrows):
        r = off + m
        if not (r0 <= r <= r0 + R - 1):
            continue
        for dr in (-1, 0, 1):
            k = m + dr
            if 0 <= k <= nrows - 1 and 0 <= off + k <= H - 1:
                tri[pbase + k, pbase + m] = 2.0 if dr == 0 else 1.0


def _matrices_for_unit(key):
    mats = {n: np.zeros((128, 128), np.float32)
            for n in ("AV", "IHF", "IHF1", "I71", "S71", "I72", "S72", "TRI")}
    if key == "stub":
        # 3 sites' bottom stubs packed at partition bases 0/34/68
        for b in range(STUB_SITES):
            _add_block(mats, b * STUB_NROWS, STUB_OFF, STUB_R0,
                       H - STUB_R0, STUB_NROWS)
    else:
        r0, R, off = _tile_geom(key)
        _add_block(mats, 0, off, r0, R, 128)
    return {k: v.astype(np.float16) for k, v in mats.items()}


def _build_weights():
    slots = {}
    packed = []
    index = {}
    for key in list(range(NFULL)) + ["stub"]:
        for name, mat in _matrices_for_unit(key).items():
            kb = mat.tobytes()
            if kb not in slots:
                slots[kb] = len(packed)
                packed.append(mat)
            index[(key, name)] = slots[kb]
    ones = np.zeros((128, 128), np.float16)
    ones[:, 0] = 1.0
    index[("ones",)] = len(packed)
    packed.append(ones)
    wts = np.concatenate(packed, axis=1)  # [128, NW*128]
    return np.ascontiguousarray(wts), index


# --------------------------------------------------------------------------
# custom DVE op: absdiff  out = |in0 - in1|, with a 2X_1PORT uop program
# --------------------------------------------------------------------------

def _register_absdiff():
    from concourse import dve_ops
    from concourse.dve_spec import Spec, Src0, Src1, maxx, lower
    from concourse.dve_uop import (
        DveOpSpec, UopConfig, UopDpConfig, AluOp, AluInp, DelayInp,
        InpSel, OutPath, OutSel,
    )
    from concourse.dve_ops import _COMPILE_CACHE

    NAME = "ABSDIFF_ANT"
    for op in dve_ops.OPS:
        if op.name == NAME:
            return op

    spec = Spec(
        body=maxx(Src0 - Src1, Src1 - Src0),
        reference=lambda in0, in1, s0, s1, imm2: np.abs(
            in0.astype(np.float32) - in1.astype(np.float32)
        ),
    )

    def build_2x(u1x):
        import copy

        u = copy.deepcopy(u1x)
        u.inp[3] = InpSel.SRC_0_HI
        u.inp[4] = InpSel.SRC_1_HI
        u.inp_enable[3] = 1
        u.inp_enable[4] = 1

        P = DelayInp.PREV_DELAY
        A = DelayInp.PREV_ALU_OUT

        def blk(op_, a, b, delay_sel, delay_en):
            d = UopDpConfig()
            d.op = op_
            d.alu_src0 = a
            d.alu_src1 = b
            d.alu_out_enable = 1
            d.delay = list(delay_sel) + [A] * (len(d.delay) - len(delay_sel))
            d.delay_enable = list(delay_en) + [0] * (
                len(d.delay_enable) - len(delay_en)
            )
            return d

        D0, D1, D2, D3 = (
            AluInp.PREV_DELAY_0, AluInp.PREV_DELAY_1,
            AluInp.PREV_DELAY_2, AluInp.PREV_DELAY_3,
        )
        ALU = AluInp.PREV_ALU_OUT
        u.datapath_config = [
            # lo: |s0-s1| on blks 0-2; hi (SRC_*_HI) on blks 3-5; the lo
            # result rides delay lane 0 to the output stage.
            blk(AluOp.SUBTRACT, D0, D1, [P, P, P, P], [1, 1, 1, 1]),
            blk(AluOp.SUBTRACT, D1, D0, [A, P, P, P], [1, 0, 1, 1]),
            blk(AluOp.MAX, D0, ALU, [P, P, P, P], [1, 0, 1, 1]),
            blk(AluOp.SUBTRACT, D2, D3, [A, P, P, P], [1, 0, 1, 1]),
            blk(AluOp.SUBTRACT, D3, D2, [P, A, P, P], [1, 1, 0, 0]),
            blk(AluOp.MAX, D1, ALU, [P, P, P, P], [1, 0, 0, 0]),
            blk(AluOp.BYPASS, ALU, ALU, [P, P, P, P], [1, 0, 0, 0]),
            blk(AluOp.BYPASS, ALU, ALU, [P, P, P, P], [1, 0, 0, 0]),
        ]
        u.out = dict(u.out)
        u.out[OutPath.WR0_LO] = OutSel.DELAY_0
        u.out[OutPath.WR0_HI] = OutSel.ALU_OUT
        u.out_enable = dict(u.out_enable)
        u.out_enable[OutPath.WR0_LO] = 1
        u.out_enable[OutPath.WR0_HI] = 1
        return u

    opcode = max(dve_ops._SUB_OPCODE_FOR_NAME.values()) + 1
    assert opcode < 0x20
    shas = {}
    specs_by_ver = {}
    for ver in ("v3", "v4"):
        uops = lower(spec, ver=ver)
        assert len(uops) == 1
        u2x = build_2x(uops[0])
        u2x.validate(ver)
        s = DveOpSpec(
            name=NAME, opcode=opcode, uops=uops, uops_2x=[u2x],
            perf_max=1, rd1_en=True,
        )
        shas[ver] = s.sha(ver)
        specs_by_ver[ver] = s

    op = dve_ops.DveOp(NAME, spec, subdim=False, uops_sha=shas)
    dve_ops.OPS.append(op)
    dve_ops._SUB_OPCODE_FOR_NAME[NAME] = opcode
    dve_ops.CUSTOM_DVE_SPECS[NAME] = spec
    # Seed the compile cache so both instruction emission and
    # dve_table_for_ops use THIS spec (with the 2x program).
    for ver, s in specs_by_ver.items():
        s.opcode = dve_ops.get_dve_sub_opcode(NAME)
        _COMPILE_CACHE[(NAME, ver)] = s
    return op


# --------------------------------------------------------------------------
# kernel build
# --------------------------------------------------------------------------

def _build(wts_np):
    import concourse.bass as bass
    import concourse.tile as tile
    from concourse import bacc, mybir

    F16 = mybir.dt.float16
    F32 = mybir.dt.float32
    AF = mybir.ActivationFunctionType
    OP = mybir.AluOpType

    absdiff = _register_absdiff()

    nc = bacc.Bacc()
    dA = nc.dram_tensor("TA", [NSITE, H, W + 2], F16, kind="ExternalInput")
    dB = nc.dram_tensor("TB", [NSITE, H, W + 2], F16, kind="ExternalInput")
    dW = nc.dram_tensor("WTS", list(wts_np.shape), F16, kind="ExternalInput")
    dO = nc.dram_tensor("OUT", [1, 1], F32, kind="ExternalOutput")

    custom_insts = []

    def absd(out_ap, in0_ap, in1_ap):
        bi = nc.vector._custom_dve(absdiff, out=out_ap, in0=in0_ap, in1=in1_ap)
        custom_insts.append(bi)
        return bi

    with tile.TileContext(nc) as tc, ExitStack() as ctx:
        persist = ctx.enter_context(tc.tile_pool(name="persist", bufs=1))
        xp = ctx.enter_context(tc.tile_pool(name="xp", bufs=6))
        fields = ctx.enter_context(tc.tile_pool(name="fields", bufs=2))
        tails = ctx.enter_context(tc.tile_pool(name="tails", bufs=3))
        ps_sml = ctx.enter_context(tc.tile_pool(name="ps_sml", bufs=1, space="PSUM"))
        ps_r = ctx.enter_context(tc.tile_pool(name="ps_r", bufs=2, space="PSUM"))

        wsb = persist.tile(list(wts_np.shape), F16)
        nc.sync.dma_start(wsb[:], dW[:])

        acc_slots = persist.tile([128, 64], F32)
        ones_f32 = persist.tile([128, 1], F32)

        oi = _WIDX[("ones",)]
        nc.vector.tensor_copy(ones_f32[:], wsb[:, oi * 128:oi * 128 + 1])

        # pre-initialize pool slots so partitions/cols never touched by DMA
        # or compute hold finite data, not virgin SBUF
        for _ in range(6):
            xu0 = xp.tile([128, 1026], F16, tag="xu")
            nc.vector.memset(xu0[0:1, :], 0.0)
        for _ in range(2):
            for tg in ("a_h", "a_1", "a_2"):
                f0 = fields.tile([128, 1026], F16, tag=tg, name=tg)
                nc.vector.memset(f0[:, 1024:1026], 0.0)

        def WT(t, name):
            i = _WIDX[(t, name)]
            return wsb[:, i * 128:(i + 1) * 128]

        def image_pipeline(dram, s, t, tag):
            x = xp.tile([128, 1026], F16, tag="x")
            if t == "stub":
                # pack 3 sites' bottom stubs: partitions 34b+m <-> site
                # 3s+b row 990+m (partitions 102..127 keep stale-but-finite
                # data; stub matrices have zero rows there)
                for b in range(STUB_SITES):
                    nc.sync.dma_start(
                        x[b * STUB_NROWS:(b + 1) * STUB_NROWS, :],
                        dram[STUB_SITES * s + b,
                             STUB_OFF:STUB_OFF + STUB_NROWS, :])
            else:
                r0, R, off = _tile_geom(t)
                nc.sync.dma_start(x[:, :], dram[s, off:off + 128, :])
            # xu[p] = image row off+p-1 (partition-shifted SBUF copy).
            # Issued from the scalar queue so the x->xu data dependency
            # doesn't stall the sync queue's x prefetch stream.
            xu = xp.tile([128, 1026], F16, tag="xu")
            nc.scalar.dma_start(xu[1:128, :], x[0:127, :])

            # |diff| fields via the fused absdiff op
            # col conventions (img col of sb col j):
            #   a_v: j   a_h: j    a_1: j    a_2: j-1
            a_v = fields.tile([128, 1024], F16, tag="a_v")
            absd(a_v[:, :], x[:, 1:1025], xu[:, 1:1025])
            a_h = fields.tile([128, 1026], F16, tag="a_h", name="a_h")
            absd(a_h[:, 0:1025], x[:, 1:1026], x[:, 0:1025])
            a_1 = fields.tile([128, 1026], F16, tag="a_1", name="a_1")
            absd(a_1[:, 0:1025], x[:, 1:1026], xu[:, 0:1025])
            a_2 = fields.tile([128, 1026], F16, tag="a_2", name="a_2")
            absd(a_2[:, 0:1025], x[:, 0:1025], xu[:, 1:1026])

            # sml assembly on PE (PSUM accumulate), 2 chunks of 512 cols
            sml = ps_sml.tile([128, 1024], F32, tag=f"sml{tag}")
            for c in range(2):
                F0 = c * 512
                o = sml[:, F0:F0 + 512]
                mm = nc.tensor.matmul
                mm(o, WT(t, "AV"), a_v[:, F0:F0 + 512], start=True, stop=False)
                mm(o, WT(t, "IHF"), a_h[:, F0:F0 + 512], start=False, stop=False)
                mm(o, WT(t, "IHF1"), a_h[:, F0 + 1:F0 + 513], start=False, stop=False)
                mm(o, WT(t, "I71"), a_1[:, F0:F0 + 512], start=False, stop=False)
                mm(o, WT(t, "S71"), a_1[:, F0 + 1:F0 + 513], start=False, stop=False)
                mm(o, WT(t, "I72"), a_2[:, F0 + 1:F0 + 513], start=False, stop=False)
                mm(o, WT(t, "S72"), a_2[:, F0:F0 + 512], start=False, stop=True)

            s2 = fields.tile([128, 1024], F16, tag=f"s2{tag}")
            nc.scalar.activation(s2[:], sml[:], AF.Square)
            return s2

        # --- tail stages, software-pipelined across iterations -----------
        def emit_q(p):
            # q = s2a - s2b with zeroed edge cols (Pool; DVE is tighter)
            q = tails.tile([128, 1026], F16, tag="q")
            nc.gpsimd.memset(q[:, 0:1], 0.0)
            nc.gpsimd.memset(q[:, 1025:1026], 0.0)
            nc.gpsimd.tensor_sub(q[:, 1:1025], p["s2a"][:], p["s2b"][:])
            p["q"] = q

        def emit_t1th(p):
            # horizontal [1,2,1] as two window-2 adds on DVE:
            # u[j] = q[j] + q[j+1]; th[c] = u[c] + u[c+1]
            q = p["q"]
            u = tails.tile([128, 1026], F16, tag="t1")
            nc.vector.tensor_add(u[:, 0:1025], q[:, 0:1025], q[:, 1:1026])
            th = tails.tile([128, 1024], F16, tag="th")
            nc.gpsimd.tensor_add(th[:], u[:, 0:1024], u[:, 1:1025])
            p["th"] = th

        def emit_trijunk(p):
            th, t, idx = p["th"], p["t"], p["idx"]
            r = ps_r.tile([128, 1024], F32, tag="r")
            for c in range(2):
                nc.tensor.matmul(
                    r[:, c * 512:(c + 1) * 512], WT(t, "TRI"),
                    th[:, c * 512:(c + 1) * 512], start=True, stop=True,
                )
            junk = tails.tile([128, 1024], F32, tag="junk")
            nc.scalar.activation(junk[:], r[:], AF.Square,
                                 accum_out=acc_slots[:, idx:idx + 1])

        # 3-stage pipeline: pipes(k) | q(k-1) | t1/th(k-1) | TRI/junk(k-2)
        # so PE's TRI and Pool's th always have a full iteration of slack.
        items = []
        units = [(s, t) for s in range(NSITE) for t in range(NFULL)]
        units += [(g, "stub") for g in range(NSITE // STUB_SITES)]
        for k, (s, t) in enumerate(units):
            s2a = image_pipeline(dA, s, t, "A")
            s2b = image_pipeline(dB, s, t, "B")
            items.append({"s2a": s2a, "s2b": s2b, "t": t, "idx": k})
            if k >= 1:
                emit_q(items[k - 1])
                emit_t1th(items[k - 1])
            if k >= 2:
                emit_trijunk(items[k - 2])
        last = len(units) - 1
        emit_q(items[last])
        emit_t1th(items[last])
        emit_trijunk(items[last - 1])
        emit_trijunk(items[last])

        tot_full = ps_r.tile([128, 1024], F32, tag="r")
        tot_ps = tot_full[0:1, 0:64]
        nc.tensor.matmul(tot_ps, ones_f32[:], acc_slots[:, 0:64],
                         start=True, stop=True)
        out_sb = persist.tile([1, 1], F32)
        nc.vector.reduce_sum(out_sb[:], tot_ps, axis=mybir.AxisListType.X)
        nc.sync.dma_start(dO[:], out_sb[:])

    for bi in custom_insts:
        bi.ins.perf_max = 1
    nc.compile()
    return nc


_WIDX = None


def _get_module():
    global _WIDX
    if "nc" in _CACHE:
        return _CACHE["nc"], _CACHE["wts"]
    wts_np, widx = _build_weights()
    _WIDX = widx
    nc = _build(wts_np)
    _CACHE["nc"] = nc
    _CACHE["wts"] = wts_np
    return nc, wts_np


def _pad_cols(a):
    # [NSITE, H, W] -> [NSITE, H, W+2] fp16 with edge-replicated columns
    out = np.empty((a.shape[0], a.shape[1], a.shape[2] + 2), np.float16)
    out[:, :, 1:-1] = a
    out[:, :, 0] = a[:, :, 0]
    out[:, :, -1] = a[:, :, -1]
    return out


def kernel(TensorA, TensorB):
    from concourse.bass_utils import run_bass_kernel_spmd

    nc, wts_np = _get_module()
    A = np.asarray(TensorA, dtype=np.float32).reshape(B * C, H, W)
    Bv = np.asarray(TensorB, dtype=np.float32).reshape(B * C, H, W)
    in_maps = []
    for c in range(NCORES):
        in_maps.append({
            "TA": _pad_cols(A[c * NSITE:(c + 1) * NSITE]),
            "TB": _pad_cols(Bv[c * NSITE:(c + 1) * NSITE]),
            "WTS": wts_np,
        })
    res = run_bass_kernel_spmd(
        nc, in_maps, core_ids=list(range(NCORES)),
        trace=bool(int(os.environ.get("CLAR_TRACE", "0"))),
    )
    _CACHE["last_results"] = res
    total = sum(float(r["OUT"][0, 0]) for r in res.results)
    return np.float32(total * FINAL_SCALE)


# revision 42
# speedup vs baseline: 1.5833x; 1.0277x over previous
"""Trainium2 Bass kernel for nn_Clar_Loss (NSML fusion-clarity MSE loss).

reference:
    x = (t+1)/2 ; s = sml(x) (8-neighbor abs-diff stencil, edge-replicate)
    nsml = G (*) s^2  (3x3 gaussian [[1,2,1],[2,4,2],[1,2,1]]/16, zero pad)
    loss = mean((nsml(A) - nsml(B))^2)

Algebra used here:
    sml((t+1)/2) = sml(t)/2          (translation invariant, pos. homogeneous)
    loss = sum((Graw (*) q)^2) / (N * 4096),  q = sA^2 - sB^2 (raw sml)
with Graw = [[1,2,1],[2,4,2],[1,2,1]] (integer), separable [1,2,1](x)[1,2,1].

Sharding: pure data-parallel over the batch dim (16 -> 2 per core); each core
returns a partial sum of (Graw(*)q)^2; host sums and rescales.

Layout: inputs are pre-padded on host to [H, W+2] (edge-replicated columns)
and pre-cast to fp16, so every row-tile is ONE full [128, 1026] fp16 DMA
issued from the SP queue (no gpsimd cast-DMA needed; xu shift-copies go on
the scalar queue so their x->xu data dependency never stalls the x prefetch
stream). Partition p <-> image row off+p with 8 full 124-row tiles per
image; the three sites' bottom 32-row stubs are PACKED into one extra tile
(partition bases 0/34/68), so each core runs 50 uniform pair-tile units
instead of 54. Vertical/partition shifts are folded into host-built 128x128
fp16 stationary matrices; image-boundary clamping and the zero padding of
s^2 are encoded as matrix edits, so no partition ever needs pad data.

All four |diff| fields are produced by a custom fused DVE absdiff op
(max(a-b, b-a)) registered with a hand-written 2X_1PORT uop program and
perf_max=1, so fp16 packed operands run at 2 elem/lane/cycle. Per pair-tile
the engines run: PE 28 assembly + 2 TRI matmuls (the saturated bottleneck,
kept gap-free so the p-state stays at full clock — warmup matmuls burn the
slow-ramp window during the first DMAs); DVE 8 absdiffs + the u window-add;
Pool q + th + edge memsets; ACT the two squares + the square-accumulate.
Tails are software-pipelined behind the next unit's field production, with
TRI/accumulate delayed a second iteration so PE never waits. Per-unit
partial sums land in a [128, 64] SBUF accumulator DMA'd out whole; the host
does the final sum and cross-core reduction.
"""

import os
from contextlib import ExitStack

import numpy as np

B, C, H, W = 16, 3, 1024, 1024
NCORES = 8
BPC = B // NCORES            # batch per core
NSITE = BPC * C              # image pairs per core
DIAG_W = 0.707
TILE_R = 124
NFULL = 8                    # full tiles 0..7 cover output rows 0..991
STUB_OFF = 990               # stub block: x rows 990..1023
STUB_R0 = 992                # stub outputs rows 992..1023 (32 rows)
STUB_NROWS = 34
STUB_SITES = 3               # sites packed per stub tile (3*34 = 102 <= 128)
FINAL_SCALE = 1.0 / (float(B * C * H * W) * 4096.0)

_CACHE = {}


def _tile_geom(t):
    """(r0, R, off): output rows [r0, r0+R), partition p <-> image row off+p."""
    r0 = t * TILE_R
    R = TILE_R
    off = 0 if t == 0 else r0 - 2
    return r0, R, off


# --------------------------------------------------------------------------
# host-built stationary matrices (lhsT layout [k, m]: out[m] += M[k,m]*in[k])
# --------------------------------------------------------------------------

def _add_block(mats, pbase, off, r0, R, nrows):
    """Emit one vertical block's stencil/tri coefficients at partition base
    ``pbase``: block partitions m=0..nrows-1 <-> image rows off+m."""
    av, ihf, ihf1 = mats["AV"], mats["IHF"], mats["IHF1"]
    i71, s71, i72, s72, tri = (
        mats["I71"], mats["S71"], mats["I72"], mats["S72"], mats["TRI"])

    def sml_valid(m):
        r = off + m
        if not (max(r0 - 1, 0) <= r <= min(r0 + R, H - 1)):
            return False
        if m > nrows - 2 and r != H - 1:
            return False
        if m < 1 and r != 0:
            return False
        return True

    for m in range(nrows):
        if not sml_valid(m):
            continue
        r = off + m
        P = pbase + m
        ihf[P, P] += 1.0           # |c-lf|
        ihf1[P, P] += 1.0          # |c-rt|
        if r >= 1:
            av[P, P] += 1.0        # |c-up| = a_v[m]
        if r <= H - 2:
            av[P + 1, P] += 1.0    # |c-dn| = a_v[m+1]
        if r == 0:                 # up-row clamps to own row
            ihf[P, P] += DIAG_W    # |c-ul| -> a_h[m, f]
            ihf1[P, P] += DIAG_W   # |c-ur| -> a_h[m, f+1]
        else:
            i71[P, P] += DIAG_W    # |c-ul| = a_1[m]
            i72[P, P] += DIAG_W    # |c-ur| = a_2[m]
        if r == H - 1:             # down-row clamps to own row
            ihf1[P, P] += DIAG_W   # |c-dr| -> a_h[m, f+1]
            ihf[P, P] += DIAG_W    # |c-dl| -> a_h[m, f]
        else:
            s71[P + 1, P] += DIAG_W  # |c-dr| = a_1[m+1] at f+1
            s72[P + 1, P] += DIAG_W  # |c-dl| = a_2[m+1] at f-1

    for m in range(nrows):
        r = off + m
        if not (r0 <= r <= r0 + R - 1):
            continue
        for dr in (-1, 0, 1):
            k = m + dr
            if 0 <= k <= nrows - 1 and 0 <= off + k <= H - 1:
                tri[pbase + k, pbase + m] = 2.0 if dr == 0 else 1.0


def _matrices_for_unit(key):
    mats = {n: np.zeros((128, 128), np.float32)
            for n in ("AV", "IHF", "IHF1", "I71", "S71", "I72", "S72", "TRI")}
    if key == "stub":
        # 3 sites' bottom stubs packed at partition bases 0/34/68
        for b in range(STUB_SITES):
            _add_block(mats, b * STUB_NROWS, STUB_OFF, STUB_R0,
                       H - STUB_R0, STUB_NROWS)
    else:
        r0, R, off = _tile_geom(key)
        _add_block(mats, 0, off, r0, R, 128)
    return {k: v.astype(np.float16) for k, v in mats.items()}


def _build_weights():
    slots = {}
    packed = []
    index = {}
    for key in list(range(NFULL)) + ["stub"]:
        for name, mat in _matrices_for_unit(key).items():
            kb = mat.tobytes()
            if kb not in slots:
                slots[kb] = len(packed)
                packed.append(mat)
            index[(key, name)] = slots[kb]
    ones = np.zeros((128, 128), np.float16)
    ones[:, 0] = 1.0
    index[("ones",)] = len(packed)
    packed.append(ones)
    wts = np.concatenate(packed, axis=1)  # [128, NW*128]
    return np.ascontiguousarray(wts), index


# --------------------------------------------------------------------------
# custom DVE op: absdiff  out = |in0 - in1|, with a 2X_1PORT uop program
# --------------------------------------------------------------------------

def _register_absdiff():
    from concourse import dve_ops
    from concourse.dve_spec import Spec, Src0, Src1, maxx, lower
    from concourse.dve_uop import (
        DveOpSpec, UopConfig, UopDpConfig, AluOp, AluInp, DelayInp,
        InpSel, OutPath, OutSel,
    )
    from concourse.dve_ops import _COMPILE_CACHE

    NAME = "ABSDIFF_ANT"
    for op in dve_ops.OPS:
        if op.name == NAME:
            return op

    spec = Spec(
        body=maxx(Src0 - Src1, Src1 - Src0),
        reference=lambda in0, in1, s0, s1, imm2: np.abs(
            in0.astype(np.float32) - in1.astype(np.float32)
        ),
    )

    def build_2x(u1x):
        import copy

        u = copy.deepcopy(u1x)
        u.inp[3] = InpSel.SRC_0_HI
        u.inp[4] = InpSel.SRC_1_HI
        u.inp_enable[3] = 1
        u.inp_enable[4] = 1

        P = DelayInp.PREV_DELAY
        A = DelayInp.PREV_ALU_OUT

        def blk(op_, a, b, delay_sel, delay_en):
            d = UopDpConfig()
            d.op = op_
            d.alu_src0 = a
            d.alu_src1 = b
            d.alu_out_enable = 1
            d.delay = list(delay_sel) + [A] * (len(d.delay) - len(delay_sel))
            d.delay_enable = list(delay_en) + [0] * (
                len(d.delay_enable) - len(delay_en)
            )
            return d

        D0, D1, D2, D3 = (
            AluInp.PREV_DELAY_0, AluInp.PREV_DELAY_1,
            AluInp.PREV_DELAY_2, AluInp.PREV_DELAY_3,
        )
        ALU = AluInp.PREV_ALU_OUT
        u.datapath_config = [
            # lo: |s0-s1| on blks 0-2; hi (SRC_*_HI) on blks 3-5; the lo
            # result rides delay lane 0 to the output stage.
            blk(AluOp.SUBTRACT, D0, D1, [P, P, P, P], [1, 1, 1, 1]),
            blk(AluOp.SUBTRACT, D1, D0, [A, P, P, P], [1, 0, 1, 1]),
            blk(AluOp.MAX, D0, ALU, [P, P, P, P], [1, 0, 1, 1]),
            blk(AluOp.SUBTRACT, D2, D3, [A, P, P, P], [1, 0, 1, 1]),
            blk(AluOp.SUBTRACT, D3, D2, [P, A, P, P], [1, 1, 0, 0]),
            blk(AluOp.MAX, D1, ALU, [P, P, P, P], [1, 0, 0, 0]),
            blk(AluOp.BYPASS, ALU, ALU, [P, P, P, P], [1, 0, 0, 0]),
            blk(AluOp.BYPASS, ALU, ALU, [P, P, P, P], [1, 0, 0, 0]),
        ]
        u.out = dict(u.out)
        u.out[OutPath.WR0_LO] = OutSel.DELAY_0
        u.out[OutPath.WR0_HI] = OutSel.ALU_OUT
        u.out_enable = dict(u.out_enable)
        u.out_enable[OutPath.WR0_LO] = 1
        u.out_enable[OutPath.WR0_HI] = 1
        return u

    opcode = max(dve_ops._SUB_OPCODE_FOR_NAME.values()) + 1
    assert opcode < 0x20
    shas = {}
    specs_by_ver = {}
    for ver in ("v3", "v4"):
        uops = lower(spec, ver=ver)
        assert len(uops) == 1
        u2x = build_2x(uops[0])
        u2x.validate(ver)
        s = DveOpSpec(
            name=NAME, opcode=opcode, uops=uops, uops_2x=[u2x],
            perf_max=1, rd1_en=True,
        )
        shas[ver] = s.sha(ver)
        specs_by_ver[ver] = s

    op = dve_ops.DveOp(NAME, spec, subdim=False, uops_sha=shas)
    dve_ops.OPS.append(op)
    dve_ops._SUB_OPCODE_FOR_NAME[NAME] = opcode
    dve_ops.CUSTOM_DVE_SPECS[NAME] = spec
    # Seed the compile cache so both instruction emission and
    # dve_table_for_ops use THIS spec (with the 2x program).
    for ver, s in specs_by_ver.items():
        s.opcode = dve_ops.get_dve_sub_opcode(NAME)
        _COMPILE_CACHE[(NAME, ver)] = s
    return op


# --------------------------------------------------------------------------
# kernel build
# --------------------------------------------------------------------------

def _build(wts_np):
    import concourse.bass as bass
    import concourse.tile as tile
    from concourse import bacc, mybir

    F16 = mybir.dt.float16
    F32 = mybir.dt.float32
    AF = mybir.ActivationFunctionType
    OP = mybir.AluOpType

    absdiff = _register_absdiff()

    nc = bacc.Bacc()
    dA = nc.dram_tensor("TA", [NSITE, H, W + 2], F16, kind="ExternalInput")
    dB = nc.dram_tensor("TB", [NSITE, H, W + 2], F16, kind="ExternalInput")
    dW = nc.dram_tensor("WTS", list(wts_np.shape), F16, kind="ExternalInput")
    dO = nc.dram_tensor("OUT", [128, 64], F32, kind="ExternalOutput")

    custom_insts = []

    def absd(out_ap, in0_ap, in1_ap):
        bi = nc.vector._custom_dve(absdiff, out=out_ap, in0=in0_ap, in1=in1_ap)
        custom_insts.append(bi)
        return bi

    with tile.TileContext(nc) as tc, ExitStack() as ctx:
        persist = ctx.enter_context(tc.tile_pool(name="persist", bufs=1))
        xp = ctx.enter_context(tc.tile_pool(name="xp", bufs=6))
        fields = ctx.enter_context(tc.tile_pool(name="fields", bufs=2))
        tails = ctx.enter_context(tc.tile_pool(name="tails", bufs=3))
        ps_sml = ctx.enter_context(tc.tile_pool(name="ps_sml", bufs=1, space="PSUM"))
        ps_r = ctx.enter_context(tc.tile_pool(name="ps_r", bufs=2, space="PSUM"))

        wsb = persist.tile(list(wts_np.shape), F16)
        # weights go on the gpsimd queue: the sync queue must start
        # streaming x tiles immediately
        nc.gpsimd.dma_start(wsb[:], dW[:])

        acc_slots = persist.tile([128, 64], F32)
        # zero the whole accumulator: unused slots would otherwise be read
        # as virgin SBUF by the final DMA-out (host sums all 128x64)
        nc.vector.memset(acc_slots[:], 0.0)

        # PE p-state warmup: junk matmuls on a zeroed tile burn the
        # slow-ramp window while the first input DMAs are in flight, so the
        # first real assembly runs at full clock.
        warm = persist.tile([128, 512], F16)
        nc.gpsimd.memset(warm[:, :], 0.0)
        warm_ps = ps_r.tile([128, 1024], F32, tag="r")
        for _ in range(14):
            nc.tensor.matmul(warm_ps[:, 0:512], warm[:, 0:128], warm[:, :],
                             start=True, stop=True)

        # pre-initialize pool slots so partitions/cols never touched by DMA
        # or compute hold finite data, not virgin SBUF
        for _ in range(6):
            xu0 = xp.tile([128, 1026], F16, tag="xu")
            nc.gpsimd.memset(xu0[0:1, :], 0.0)
        for _ in range(2):
            for tg in ("a_h", "a_1", "a_2"):
                f0 = fields.tile([128, 1026], F16, tag=tg, name=tg)
                nc.gpsimd.memset(f0[:, 1024:1026], 0.0)

        def WT(t, name):
            i = _WIDX[(t, name)]
            return wsb[:, i * 128:(i + 1) * 128]

        def image_pipeline(dram, s, t, tag):
            x = xp.tile([128, 1026], F16, tag="x")
            if t == "stub":
                # pack 3 sites' bottom stubs: partitions 34b+m <-> site
                # 3s+b row 990+m (partitions 102..127 keep stale-but-finite
                # data; stub matrices have zero rows there)
                for b in range(STUB_SITES):
                    nc.sync.dma_start(
                        x[b * STUB_NROWS:(b + 1) * STUB_NROWS, :],
                        dram[STUB_SITES * s + b,
                             STUB_OFF:STUB_OFF + STUB_NROWS, :])
            else:
                r0, R, off = _tile_geom(t)
                nc.sync.dma_start(x[:, :], dram[s, off:off + 128, :])
            # xu[p] = image row off+p-1 (partition-shifted SBUF copy).
            # Issued from the scalar queue so the x->xu data dependency
            # doesn't stall the sync queue's x prefetch stream.
            xu = xp.tile([128, 1026], F16, tag="xu")
            nc.scalar.dma_start(xu[1:128, :], x[0:127, :])

            # |diff| fields via the fused absdiff op
            # col conventions (img col of sb col j):
            #   a_v: j   a_h: j    a_1: j    a_2: j-1
            a_v = fields.tile([128, 1024], F16, tag="a_v")
            absd(a_v[:, :], x[:, 1:1025], xu[:, 1:1025])
            a_h = fields.tile([128, 1026], F16, tag="a_h", name="a_h")
            absd(a_h[:, 0:1025], x[:, 1:1026], x[:, 0:1025])
            a_1 = fields.tile([128, 1026], F16, tag="a_1", name="a_1")
            absd(a_1[:, 0:1025], x[:, 1:1026], xu[:, 0:1025])
            a_2 = fields.tile([128, 1026], F16, tag="a_2", name="a_2")
            absd(a_2[:, 0:1025], x[:, 0:1025], xu[:, 1:1026])

            # sml assembly on PE (PSUM accumulate), 2 chunks of 512 cols
            sml = ps_sml.tile([128, 1024], F32, tag=f"sml{tag}")
            for c in range(2):
                F0 = c * 512
                o = sml[:, F0:F0 + 512]
                mm = nc.tensor.matmul
                mm(o, WT(t, "AV"), a_v[:, F0:F0 + 512], start=True, stop=False)
                mm(o, WT(t, "IHF"), a_h[:, F0:F0 + 512], start=False, stop=False)
                mm(o, WT(t, "IHF1"), a_h[:, F0 + 1:F0 + 513], start=False, stop=False)
                mm(o, WT(t, "I71"), a_1[:, F0:F0 + 512], start=False, stop=False)
                mm(o, WT(t, "S71"), a_1[:, F0 + 1:F0 + 513], start=False, stop=False)
                mm(o, WT(t, "I72"), a_2[:, F0 + 1:F0 + 513], start=False, stop=False)
                mm(o, WT(t, "S72"), a_2[:, F0:F0 + 512], start=False, stop=True)

            s2 = fields.tile([128, 1024], F16, tag=f"s2{tag}")
            nc.scalar.activation(s2[:], sml[:], AF.Square)
            return s2

        # --- tail stages, software-pipelined across iterations -----------
        def emit_q(p):
            # q = s2a - s2b with zeroed edge cols (Pool; DVE is tighter)
            q = tails.tile([128, 1026], F16, tag="q")
            nc.gpsimd.memset(q[:, 0:1], 0.0)
            nc.gpsimd.memset(q[:, 1025:1026], 0.0)
            nc.gpsimd.tensor_sub(q[:, 1:1025], p["s2a"][:], p["s2b"][:])
            p["q"] = q

        def emit_t1th(p):
            # horizontal [1,2,1] as two window-2 adds on DVE:
            # u[j] = q[j] + q[j+1]; th[c] = u[c] + u[c+1]
            q = p["q"]
            u = tails.tile([128, 1026], F16, tag="t1")
            nc.vector.tensor_add(u[:, 0:1025], q[:, 0:1025], q[:, 1:1026])
            th = tails.tile([128, 1024], F16, tag="th")
            nc.gpsimd.tensor_add(th[:], u[:, 0:1024], u[:, 1:1025])
            p["th"] = th

        def emit_trijunk(p, on_dve=False):
            th, t, idx = p["th"], p["t"], p["idx"]
            r = ps_r.tile([128, 1024], F32, tag="r")
            for c in range(2):
                nc.tensor.matmul(
                    r[:, c * 512:(c + 1) * 512], WT(t, "TRI"),
                    th[:, c * 512:(c + 1) * 512], start=True, stop=True,
                )
            junk = tails.tile([128, 1024], F32, tag="junk")
            if on_dve:
                # drain only: square+accumulate on DVE so the last two
                # accumulations run in parallel on ACT and DVE
                nc.vector.affine_mul_reduce(
                    junk[:], acc_slots[:, idx:idx + 1], r[:], r[:], 1.0, 0.0)
            else:
                nc.scalar.activation(junk[:], r[:], AF.Square,
                                     accum_out=acc_slots[:, idx:idx + 1])

        # 3-stage pipeline: pipes(k) | q(k-1) | t1/th(k-1) | TRI/junk(k-2)
        # so PE's TRI and Pool's th always have a full iteration of slack.
        items = []
        units = [(s, t) for s in range(NSITE) for t in range(NFULL)]
        units += [(g, "stub") for g in range(NSITE // STUB_SITES)]
        for k, (s, t) in enumerate(units):
            s2a = image_pipeline(dA, s, t, "A")
            s2b = image_pipeline(dB, s, t, "B")
            items.append({"s2a": s2a, "s2b": s2b, "t": t, "idx": k})
            if k >= 1 and k - 1 < len(units) - 2:
                emit_q(items[k - 1])
                emit_t1th(items[k - 1])
            if k >= 2:
                emit_trijunk(items[k - 2])
        # drain fast-path: run the final two tails' horizontal stage on DVE
        # (short chain) so the kernel doesn't trail off on the slow Pool ops
        def fast_tail(p):
            q = tails.tile([128, 1026], F16, tag="q")
            nc.vector.memset(q[:, 0:1], 0.0)
            nc.vector.memset(q[:, 1025:1026], 0.0)
            nc.vector.tensor_sub(q[:, 1:1025], p["s2a"][:], p["s2b"][:])
            u = tails.tile([128, 1026], F16, tag="t1")
            nc.vector.tensor_add(u[:, 0:1025], q[:, 0:1025], q[:, 1:1026])
            th = tails.tile([128, 1024], F16, tag="th")
            nc.vector.tensor_add(th[:], u[:, 0:1024], u[:, 1:1025])
            p["th"] = th

        last = len(units) - 1
        fast_tail(items[last - 1])
        fast_tail(items[last])
        emit_trijunk(items[last - 1])
        emit_trijunk(items[last])

        # ship the [128, 64] partial-accumulator tile; the host sums it
        # together with the cross-core reduction
        nc.sync.dma_start(dO[:], acc_slots[:])

    for bi in custom_insts:
        bi.ins.perf_max = 1
    nc.compile()
    return nc


_WIDX = None


def _get_module():
    global _WIDX
    if "nc" in _CACHE:
        return _CACHE["nc"], _CACHE["wts"]
    wts_np, widx = _build_weights()
    _WIDX = widx
    nc = _build(wts_np)
    _CACHE["nc"] = nc
    _CACHE["wts"] = wts_np
    return nc, wts_np


def _pad_cols(a):
    # [NSITE, H, W] -> [NSITE, H, W+2] fp16 with edge-replicated columns
    out = np.empty((a.shape[0], a.shape[1], a.shape[2] + 2), np.float16)
    out[:, :, 1:-1] = a
    out[:, :, 0] = a[:, :, 0]
    out[:, :, -1] = a[:, :, -1]
    return out


def kernel(TensorA, TensorB):
    from concourse.bass_utils import run_bass_kernel_spmd

    nc, wts_np = _get_module()
    A = np.asarray(TensorA, dtype=np.float32).reshape(B * C, H, W)
    Bv = np.asarray(TensorB, dtype=np.float32).reshape(B * C, H, W)
    in_maps = []
    for c in range(NCORES):
        in_maps.append({
            "TA": _pad_cols(A[c * NSITE:(c + 1) * NSITE]),
            "TB": _pad_cols(Bv[c * NSITE:(c + 1) * NSITE]),
            "WTS": wts_np,
        })
    res = run_bass_kernel_spmd(
        nc, in_maps, core_ids=list(range(NCORES)),
        trace=bool(int(os.environ.get("CLAR_TRACE", "0"))),
    )
    _CACHE["last_results"] = res
    total = sum(float(r["OUT"].astype(np.float64).sum()) for r in res.results)
    return np.float32(total * FINAL_SCALE)


# revision 43
# speedup vs baseline: 1.5837x; 1.0002x over previous
"""Trainium2 Bass kernel for nn_Clar_Loss (NSML fusion-clarity MSE loss).

reference:
    x = (t+1)/2 ; s = sml(x) (8-neighbor abs-diff stencil, edge-replicate)
    nsml = G (*) s^2  (3x3 gaussian [[1,2,1],[2,4,2],[1,2,1]]/16, zero pad)
    loss = mean((nsml(A) - nsml(B))^2)

Algebra used here:
    sml((t+1)/2) = sml(t)/2          (translation invariant, pos. homogeneous)
    loss = sum((Graw (*) q)^2) / (N * 4096),  q = sA^2 - sB^2 (raw sml)
with Graw = [[1,2,1],[2,4,2],[1,2,1]] (integer), separable [1,2,1](x)[1,2,1].

Sharding: pure data-parallel over the batch dim (16 -> 2 per core); each core
returns a partial sum of (Graw(*)q)^2; host sums and rescales.

Layout: inputs are pre-padded on host to [H, W+2] (edge-replicated columns)
and pre-cast to fp16, so every row-tile is ONE full [128, 1026] fp16 DMA
issued from the SP queue (no gpsimd cast-DMA needed; xu shift-copies go on
the scalar queue so their x->xu data dependency never stalls the x prefetch
stream). Partition p <-> image row off+p with 8 full 124-row tiles per
image; the three sites' bottom 32-row stubs are PACKED into one extra tile
(partition bases 0/34/68), so each core runs 50 uniform pair-tile units
instead of 54. Vertical/partition shifts are folded into host-built 128x128
fp16 stationary matrices; image-boundary clamping and the zero padding of
s^2 are encoded as matrix edits, so no partition ever needs pad data.

All four |diff| fields are produced by a custom fused DVE absdiff op
(max(a-b, b-a)) registered with a hand-written 2X_1PORT uop program and
perf_max=1, so fp16 packed operands run at 2 elem/lane/cycle. Per pair-tile
the engines run: PE 28 assembly + 2 TRI matmuls (the saturated bottleneck,
kept gap-free so the p-state stays at full clock — warmup matmuls burn the
slow-ramp window during the first DMAs); DVE 8 absdiffs + the u window-add;
Pool q + th + edge memsets; ACT the two squares + the square-accumulate.
Tails are software-pipelined behind the next unit's field production, with
TRI/accumulate delayed a second iteration so PE never waits. Per-unit
partial sums land in a [128, 64] SBUF accumulator DMA'd out whole; the host
does the final sum and cross-core reduction.
"""

import os
from contextlib import ExitStack

import numpy as np

B, C, H, W = 16, 3, 1024, 1024
NCORES = 8
BPC = B // NCORES            # batch per core
NSITE = BPC * C              # image pairs per core
DIAG_W = 0.707
TILE_R = 124
NFULL = 8                    # full tiles 0..7 cover output rows 0..991
STUB_OFF = 990               # stub block: x rows 990..1023
STUB_R0 = 992                # stub outputs rows 992..1023 (32 rows)
STUB_NROWS = 34
STUB_SITES = 3               # sites packed per stub tile (3*34 = 102 <= 128)
FINAL_SCALE = 1.0 / (float(B * C * H * W) * 4096.0)

_CACHE = {}


def _tile_geom(t):
    """(r0, R, off): output rows [r0, r0+R), partition p <-> image row off+p."""
    r0 = t * TILE_R
    R = TILE_R
    off = 0 if t == 0 else r0 - 2
    return r0, R, off


# --------------------------------------------------------------------------
# host-built stationary matrices (lhsT layout [k, m]: out[m] += M[k,m]*in[k])
# --------------------------------------------------------------------------

def _add_block(mats, pbase, off, r0, R, nrows):
    """Emit one vertical block's stencil/tri coefficients at partition base
    ``pbase``: block partitions m=0..nrows-1 <-> image rows off+m."""
    av, ihf, ihf1 = mats["AV"], mats["IHF"], mats["IHF1"]
    i71, s71, i72, s72, tri = (
        mats["I71"], mats["S71"], mats["I72"], mats["S72"], mats["TRI"])

    def sml_valid(m):
        r = off + m
        if not (max(r0 - 1, 0) <= r <= min(r0 + R, H - 1)):
            return False
        if m > nrows - 2 and r != H - 1:
            return False
        if m < 1 and r != 0:
            return False
        return True

    for m in range(nrows):
        if not sml_valid(m):
            continue
        r = off + m
        P = pbase + m
        ihf[P, P] += 1.0           # |c-lf|
        ihf1[P, P] += 1.0          # |c-rt|
        if r >= 1:
            av[P, P] += 1.0        # |c-up| = a_v[m]
        if r <= H - 2:
            av[P + 1, P] += 1.0    # |c-dn| = a_v[m+1]
        if r == 0:                 # up-row clamps to own row
            ihf[P, P] += DIAG_W    # |c-ul| -> a_h[m, f]
            ihf1[P, P] += DIAG_W   # |c-ur| -> a_h[m, f+1]
        else:
            i71[P, P] += DIAG_W    # |c-ul| = a_1[m]
            i72[P, P] += DIAG_W    # |c-ur| = a_2[m]
        if r == H - 1:             # down-row clamps to own row
            ihf1[P, P] += DIAG_W   # |c-dr| -> a_h[m, f+1]
            ihf[P, P] += DIAG_W    # |c-dl| -> a_h[m, f]
        else:
            s71[P + 1, P] += DIAG_W  # |c-dr| = a_1[m+1] at f+1
            s72[P + 1, P] += DIAG_W  # |c-dl| = a_2[m+1] at f-1

    for m in range(nrows):
        r = off + m
        if not (r0 <= r <= r0 + R - 1):
            continue
        for dr in (-1, 0, 1):
            k = m + dr
            if 0 <= k <= nrows - 1 and 0 <= off + k <= H - 1:
                tri[pbase + k, pbase + m] = 2.0 if dr == 0 else 1.0


def _matrices_for_unit(key):
    mats = {n: np.zeros((128, 128), np.float32)
            for n in ("AV", "IHF", "IHF1", "I71", "S71", "I72", "S72", "TRI")}
    if key == "stub":
        # 3 sites' bottom stubs packed at partition bases 0/34/68
        for b in range(STUB_SITES):
            _add_block(mats, b * STUB_NROWS, STUB_OFF, STUB_R0,
                       H - STUB_R0, STUB_NROWS)
    else:
        r0, R, off = _tile_geom(key)
        _add_block(mats, 0, off, r0, R, 128)
    return {k: v.astype(np.float16) for k, v in mats.items()}


def _build_weights():
    slots = {}
    packed = []
    index = {}
    for key in list(range(NFULL)) + ["stub"]:
        for name, mat in _matrices_for_unit(key).items():
            kb = mat.tobytes()
            if kb not in slots:
                slots[kb] = len(packed)
                packed.append(mat)
            index[(key, name)] = slots[kb]
    ones = np.zeros((128, 128), np.float16)
    ones[:, 0] = 1.0
    index[("ones",)] = len(packed)
    packed.append(ones)
    wts = np.concatenate(packed, axis=1)  # [128, NW*128]
    return np.ascontiguousarray(wts), index


# --------------------------------------------------------------------------
# custom DVE op: absdiff  out = |in0 - in1|, with a 2X_1PORT uop program
# --------------------------------------------------------------------------

def _register_absdiff():
    from concourse import dve_ops
    from concourse.dve_spec import Spec, Src0, Src1, maxx, lower
    from concourse.dve_uop import (
        DveOpSpec, UopConfig, UopDpConfig, AluOp, AluInp, DelayInp,
        InpSel, OutPath, OutSel,
    )
    from concourse.dve_ops import _COMPILE_CACHE

    NAME = "ABSDIFF_ANT"
    for op in dve_ops.OPS:
        if op.name == NAME:
            return op

    spec = Spec(
        body=maxx(Src0 - Src1, Src1 - Src0),
        reference=lambda in0, in1, s0, s1, imm2: np.abs(
            in0.astype(np.float32) - in1.astype(np.float32)
        ),
    )

    def build_2x(u1x):
        import copy

        u = copy.deepcopy(u1x)
        u.inp[3] = InpSel.SRC_0_HI
        u.inp[4] = InpSel.SRC_1_HI
        u.inp_enable[3] = 1
        u.inp_enable[4] = 1

        P = DelayInp.PREV_DELAY
        A = DelayInp.PREV_ALU_OUT

        def blk(op_, a, b, delay_sel, delay_en):
            d = UopDpConfig()
            d.op = op_
            d.alu_src0 = a
            d.alu_src1 = b
            d.alu_out_enable = 1
            d.delay = list(delay_sel) + [A] * (len(d.delay) - len(delay_sel))
            d.delay_enable = list(delay_en) + [0] * (
                len(d.delay_enable) - len(delay_en)
            )
            return d

        D0, D1, D2, D3 = (
            AluInp.PREV_DELAY_0, AluInp.PREV_DELAY_1,
            AluInp.PREV_DELAY_2, AluInp.PREV_DELAY_3,
        )
        ALU = AluInp.PREV_ALU_OUT
        u.datapath_config = [
            # lo: |s0-s1| on blks 0-2; hi (SRC_*_HI) on blks 3-5; the lo
            # result rides delay lane 0 to the output stage.
            blk(AluOp.SUBTRACT, D0, D1, [P, P, P, P], [1, 1, 1, 1]),
            blk(AluOp.SUBTRACT, D1, D0, [A, P, P, P], [1, 0, 1, 1]),
            blk(AluOp.MAX, D0, ALU, [P, P, P, P], [1, 0, 1, 1]),
            blk(AluOp.SUBTRACT, D2, D3, [A, P, P, P], [1, 0, 1, 1]),
            blk(AluOp.SUBTRACT, D3, D2, [P, A, P, P], [1, 1, 0, 0]),
            blk(AluOp.MAX, D1, ALU, [P, P, P, P], [1, 0, 0, 0]),
            blk(AluOp.BYPASS, ALU, ALU, [P, P, P, P], [1, 0, 0, 0]),
            blk(AluOp.BYPASS, ALU, ALU, [P, P, P, P], [1, 0, 0, 0]),
        ]
        u.out = dict(u.out)
        u.out[OutPath.WR0_LO] = OutSel.DELAY_0
        u.out[OutPath.WR0_HI] = OutSel.ALU_OUT
        u.out_enable = dict(u.out_enable)
        u.out_enable[OutPath.WR0_LO] = 1
        u.out_enable[OutPath.WR0_HI] = 1
        return u

    opcode = max(dve_ops._SUB_OPCODE_FOR_NAME.values()) + 1
    assert opcode < 0x20
    shas = {}
    specs_by_ver = {}
    for ver in ("v3", "v4"):
        uops = lower(spec, ver=ver)
        assert len(uops) == 1
        u2x = build_2x(uops[0])
        u2x.validate(ver)
        s = DveOpSpec(
            name=NAME, opcode=opcode, uops=uops, uops_2x=[u2x],
            perf_max=1, rd1_en=True,
        )
        shas[ver] = s.sha(ver)
        specs_by_ver[ver] = s

    op = dve_ops.DveOp(NAME, spec, subdim=False, uops_sha=shas)
    dve_ops.OPS.append(op)
    dve_ops._SUB_OPCODE_FOR_NAME[NAME] = opcode
    dve_ops.CUSTOM_DVE_SPECS[NAME] = spec
    # Seed the compile cache so both instruction emission and
    # dve_table_for_ops use THIS spec (with the 2x program).
    for ver, s in specs_by_ver.items():
        s.opcode = dve_ops.get_dve_sub_opcode(NAME)
        _COMPILE_CACHE[(NAME, ver)] = s
    return op


# --------------------------------------------------------------------------
# kernel build
# --------------------------------------------------------------------------

def _build(wts_np):
    import concourse.bass as bass
    import concourse.tile as tile
    from concourse import bacc, mybir

    F16 = mybir.dt.float16
    F32 = mybir.dt.float32
    AF = mybir.ActivationFunctionType
    OP = mybir.AluOpType

    absdiff = _register_absdiff()

    nc = bacc.Bacc()
    dA = nc.dram_tensor("TA", [NSITE, H, W + 2], F16, kind="ExternalInput")
    dB = nc.dram_tensor("TB", [NSITE, H, W + 2], F16, kind="ExternalInput")
    dW = nc.dram_tensor("WTS", list(wts_np.shape), F16, kind="ExternalInput")
    dO = nc.dram_tensor("OUT", [128, 64], F32, kind="ExternalOutput")

    custom_insts = []

    def absd(out_ap, in0_ap, in1_ap):
        bi = nc.vector._custom_dve(absdiff, out=out_ap, in0=in0_ap, in1=in1_ap)
        custom_insts.append(bi)
        return bi

    with tile.TileContext(nc) as tc, ExitStack() as ctx:
        persist = ctx.enter_context(tc.tile_pool(name="persist", bufs=1))
        xp = ctx.enter_context(tc.tile_pool(name="xp", bufs=6))
        fields = ctx.enter_context(tc.tile_pool(name="fields", bufs=2))
        tails = ctx.enter_context(tc.tile_pool(name="tails", bufs=3))
        ps_sml = ctx.enter_context(tc.tile_pool(name="ps_sml", bufs=1, space="PSUM"))
        ps_r = ctx.enter_context(tc.tile_pool(name="ps_r", bufs=2, space="PSUM"))

        wsb = persist.tile(list(wts_np.shape), F16)
        # weights go on the gpsimd queue: the sync queue must start
        # streaming x tiles immediately
        nc.gpsimd.dma_start(wsb[:], dW[:])

        acc_slots = persist.tile([128, 64], F32)
        # zero the whole accumulator: unused slots would otherwise be read
        # as virgin SBUF by the final DMA-out (host sums all 128x64)
        nc.vector.memset(acc_slots[:], 0.0)

        # PE p-state warmup: junk matmuls on a zeroed tile burn the
        # slow-ramp window while the first input DMAs are in flight, so the
        # first real assembly runs at full clock.
        warm = persist.tile([128, 512], F16)
        nc.gpsimd.memset(warm[:, :], 0.0)
        warm_ps = ps_r.tile([128, 1024], F32, tag="r")
        for _ in range(14):
            nc.tensor.matmul(warm_ps[:, 0:512], warm[:, 0:128], warm[:, :],
                             start=True, stop=True)

        # pre-initialize pool slots so partitions/cols never touched by DMA
        # or compute hold finite data, not virgin SBUF
        for _ in range(6):
            xu0 = xp.tile([128, 1026], F16, tag="xu")
            nc.gpsimd.memset(xu0[0:1, :], 0.0)
        for _ in range(2):
            for tg in ("a_h", "a_1", "a_2"):
                f0 = fields.tile([128, 1026], F16, tag=tg, name=tg)
                nc.gpsimd.memset(f0[:, 1024:1026], 0.0)

        def WT(t, name):
            i = _WIDX[(t, name)]
            return wsb[:, i * 128:(i + 1) * 128]

        def image_pipeline(dram, s, t, tag):
            x = xp.tile([128, 1026], F16, tag="x")
            if t == "stub":
                # pack 3 sites' bottom stubs: partitions 34b+m <-> site
                # 3s+b row 990+m (partitions 102..127 keep stale-but-finite
                # data; stub matrices have zero rows there)
                for b in range(STUB_SITES):
                    nc.sync.dma_start(
                        x[b * STUB_NROWS:(b + 1) * STUB_NROWS, :],
                        dram[STUB_SITES * s + b,
                             STUB_OFF:STUB_OFF + STUB_NROWS, :])
            else:
                r0, R, off = _tile_geom(t)
                nc.sync.dma_start(x[:, :], dram[s, off:off + 128, :])
            # xu[p] = image row off+p-1 (partition-shifted SBUF copy).
            # Issued from the scalar queue so the x->xu data dependency
            # doesn't stall the sync queue's x prefetch stream.
            xu = xp.tile([128, 1026], F16, tag="xu")
            nc.scalar.dma_start(xu[1:128, :], x[0:127, :])

            # |diff| fields via the fused absdiff op
            # col conventions (img col of sb col j):
            #   a_v: j   a_h: j    a_1: j    a_2: j-1
            a_v = fields.tile([128, 1024], F16, tag="a_v")
            absd(a_v[:, :], x[:, 1:1025], xu[:, 1:1025])
            a_h = fields.tile([128, 1026], F16, tag="a_h", name="a_h")
            absd(a_h[:, 0:1025], x[:, 1:1026], x[:, 0:1025])
            a_1 = fields.tile([128, 1026], F16, tag="a_1", name="a_1")
            absd(a_1[:, 0:1025], x[:, 1:1026], xu[:, 0:1025])
            a_2 = fields.tile([128, 1026], F16, tag="a_2", name="a_2")
            absd(a_2[:, 0:1025], x[:, 0:1025], xu[:, 1:1026])

            # sml assembly on PE (PSUM accumulate), 2 chunks of 512 cols
            sml = ps_sml.tile([128, 1024], F32, tag=f"sml{tag}")
            for c in range(2):
                F0 = c * 512
                o = sml[:, F0:F0 + 512]
                mm = nc.tensor.matmul
                mm(o, WT(t, "AV"), a_v[:, F0:F0 + 512], start=True, stop=False)
                mm(o, WT(t, "IHF"), a_h[:, F0:F0 + 512], start=False, stop=False)
                mm(o, WT(t, "IHF1"), a_h[:, F0 + 1:F0 + 513], start=False, stop=False)
                mm(o, WT(t, "I71"), a_1[:, F0:F0 + 512], start=False, stop=False)
                mm(o, WT(t, "S71"), a_1[:, F0 + 1:F0 + 513], start=False, stop=False)
                mm(o, WT(t, "I72"), a_2[:, F0 + 1:F0 + 513], start=False, stop=False)
                mm(o, WT(t, "S72"), a_2[:, F0:F0 + 512], start=False, stop=True)

            s2 = fields.tile([128, 1024], F16, tag=f"s2{tag}")
            nc.scalar.activation(s2[:], sml[:], AF.Square)
            return s2

        # --- tail stages, software-pipelined across iterations -----------
        def emit_q(p):
            # q = s2a - s2b with zeroed edge cols (Pool; DVE is tighter)
            q = tails.tile([128, 1026], F16, tag="q")
            nc.gpsimd.memset(q[:, 0:1], 0.0)
            nc.gpsimd.memset(q[:, 1025:1026], 0.0)
            nc.gpsimd.tensor_sub(q[:, 1:1025], p["s2a"][:], p["s2b"][:])
            p["q"] = q

        def emit_t1th(p):
            # horizontal [1,2,1] as two window-2 adds on DVE:
            # u[j] = q[j] + q[j+1]; th[c] = u[c] + u[c+1]
            q = p["q"]
            u = tails.tile([128, 1026], F16, tag="t1")
            nc.vector.tensor_add(u[:, 0:1025], q[:, 0:1025], q[:, 1:1026])
            th = tails.tile([128, 1024], F16, tag="th")
            nc.gpsimd.tensor_add(th[:], u[:, 0:1024], u[:, 1:1025])
            p["th"] = th

        def emit_trijunk(p, on_dve=False):
            th, t, idx = p["th"], p["t"], p["idx"]
            r = ps_r.tile([128, 1024], F32, tag="r")
            for c in range(2):
                nc.tensor.matmul(
                    r[:, c * 512:(c + 1) * 512], WT(t, "TRI"),
                    th[:, c * 512:(c + 1) * 512], start=True, stop=True,
                )
            junk = tails.tile([128, 1024], F32, tag="junk")
            if on_dve:
                # drain only: square+accumulate on DVE so the last two
                # accumulations run in parallel on ACT and DVE
                nc.vector.affine_mul_reduce(
                    junk[:], acc_slots[:, idx:idx + 1], r[:], r[:], 1.0, 0.0)
            else:
                nc.scalar.activation(junk[:], r[:], AF.Square,
                                     accum_out=acc_slots[:, idx:idx + 1])

        # 3-stage pipeline: pipes(k) | q(k-1) | t1/th(k-1) | TRI/junk(k-2)
        # so PE's TRI and Pool's th always have a full iteration of slack.
        items = []
        units = [(s, t) for s in range(NSITE) for t in range(NFULL)]
        units += [(g, "stub") for g in range(NSITE // STUB_SITES)]
        for k, (s, t) in enumerate(units):
            s2a = image_pipeline(dA, s, t, "A")
            s2b = image_pipeline(dB, s, t, "B")
            items.append({"s2a": s2a, "s2b": s2b, "t": t, "idx": k})
            if k >= 1 and k - 1 < len(units) - 2:
                emit_q(items[k - 1])
                emit_t1th(items[k - 1])
            if k >= 2:
                emit_trijunk(items[k - 2])
        # drain fast-path: run the final two tails' horizontal stage on DVE
        # (short chain) so the kernel doesn't trail off on the slow Pool ops
        def fast_tail(p):
            q = tails.tile([128, 1026], F16, tag="q")
            nc.vector.memset(q[:, 0:1], 0.0)
            nc.vector.memset(q[:, 1025:1026], 0.0)
            nc.vector.tensor_sub(q[:, 1:1025], p["s2a"][:], p["s2b"][:])
            u = tails.tile([128, 1026], F16, tag="t1")
            nc.vector.tensor_add(u[:, 0:1025], q[:, 0:1025], q[:, 1:1026])
            th = tails.tile([128, 1024], F16, tag="th")
            nc.vector.tensor_add(th[:], u[:, 0:1024], u[:, 1:1025])
            p["th"] = th

        last = len(units) - 1
        fast_tail(items[last - 1])
        fast_tail(items[last])
        emit_trijunk(items[last - 1])
        emit_trijunk(items[last])

        # ship the [128, 64] partial-accumulator tile; the host sums it
        # together with the cross-core reduction. The bulk goes out as soon
        # as units 0..last-2 have accumulated; only a 2-column tail DMA
        # waits on the final two units.
        nlast = len(units) - 2
        nc.sync.dma_start(dO[:, 0:nlast], acc_slots[:, 0:nlast])
        nc.sync.dma_start(dO[:, nlast:64], acc_slots[:, nlast:64])

    for bi in custom_insts:
        bi.ins.perf_max = 1
    nc.compile()
    return nc


_WIDX = None


def _get_module():
    global _WIDX
    if "nc" in _CACHE:
        return _CACHE["nc"], _CACHE["wts"]
    wts_np, widx = _build_weights()
    _WIDX = widx
    nc = _build(wts_np)
    _CACHE["nc"] = nc
    _CACHE["wts"] = wts_np
    return nc, wts_np


def _pad_cols(a):
    # [NSITE, H, W] -> [NSITE, H, W+2] fp16 with edge-replicated columns
    out = np.empty((a.shape[0], a.shape[1], a.shape[2] + 2), np.float16)
    out[:, :, 1:-1] = a
    out[:, :, 0] = a[:, :, 0]
    out[:, :, -1] = a[:, :, -1]
    return out


def kernel(TensorA, TensorB):
    from concourse.bass_utils import run_bass_kernel_spmd

    nc, wts_np = _get_module()
    A = np.asarray(TensorA, dtype=np.float32).reshape(B * C, H, W)
    Bv = np.asarray(TensorB, dtype=np.float32).reshape(B * C, H, W)
    in_maps = []
    for c in range(NCORES):
        in_maps.append({
            "TA": _pad_cols(A[c * NSITE:(c + 1) * NSITE]),
            "TB": _pad_cols(Bv[c * NSITE:(c + 1) * NSITE]),
            "WTS": wts_np,
        })
    res = run_bass_kernel_spmd(
        nc, in_maps, core_ids=list(range(NCORES)),
        trace=bool(int(os.environ.get("CLAR_TRACE", "0"))),
    )
    _CACHE["last_results"] = res
    total = sum(float(r["OUT"].astype(np.float64).sum()) for r in res.results)
    return np.float32(total * FINAL_SCALE)


# revision 46
# speedup vs baseline: 1.5851x; 1.0009x over previous
"""Trainium2 Bass kernel for nn_Clar_Loss (NSML fusion-clarity MSE loss).

reference:
    x = (t+1)/2 ; s = sml(x) (8-neighbor abs-diff stencil, edge-replicate)
    nsml = G (*) s^2  (3x3 gaussian [[1,2,1],[2,4,2],[1,2,1]]/16, zero pad)
    loss = mean((nsml(A) - nsml(B))^2)

Algebra used here:
    sml((t+1)/2) = sml(t)/2          (translation invariant, pos. homogeneous)
    loss = sum((Graw (*) q)^2) / (N * 4096),  q = sA^2 - sB^2 (raw sml)
with Graw = [[1,2,1],[2,4,2],[1,2,1]] (integer), separable [1,2,1](x)[1,2,1].

Sharding: pure data-parallel over the batch dim (16 -> 2 per core); each core
returns a partial sum of (Graw(*)q)^2; host sums and rescales.

Layout: inputs are pre-padded on host to [H, W+2] (edge-replicated columns)
and pre-cast to fp16, so every row-tile is ONE full [128, 1026] fp16 DMA
issued from the SP queue (no gpsimd cast-DMA needed; xu shift-copies go on
the scalar queue so their x->xu data dependency never stalls the x prefetch
stream). Partition p <-> image row off+p with 8 full 124-row tiles per
image; the three sites' bottom 32-row stubs are PACKED into one extra tile
(partition bases 0/34/68), so each core runs 50 uniform pair-tile units
instead of 54. Vertical/partition shifts are folded into host-built 128x128
fp16 stationary matrices; image-boundary clamping and the zero padding of
s^2 are encoded as matrix edits, so no partition ever needs pad data.

All four |diff| fields are produced by a custom fused DVE absdiff op
(max(a-b, b-a)) registered with a hand-written 2X_1PORT uop program and
perf_max=1, so fp16 packed operands run at 2 elem/lane/cycle. Per pair-tile
the engines run: PE 28 assembly + 2 TRI matmuls (the saturated bottleneck,
kept gap-free so the p-state stays at full clock — warmup matmuls burn the
slow-ramp window during the first DMAs); DVE 8 absdiffs + the u window-add;
Pool q + th + edge memsets; ACT the two squares + the square-accumulate.
Tails are software-pipelined behind the next unit's field production, with
TRI/accumulate delayed a second iteration so PE never waits. Per-unit
partial sums land in a [128, 64] SBUF accumulator DMA'd out whole; the host
does the final sum and cross-core reduction.
"""

import os
from contextlib import ExitStack

import numpy as np

B, C, H, W = 16, 3, 1024, 1024
NCORES = 8
BPC = B // NCORES            # batch per core
NSITE = BPC * C              # image pairs per core
DIAG_W = 0.707
TILE_R = 124
NFULL = 8                    # full tiles 0..7 cover output rows 0..991
STUB_OFF = 990               # stub block: x rows 990..1023
STUB_R0 = 992                # stub outputs rows 992..1023 (32 rows)
STUB_NROWS = 34
STUB_SITES = 3               # sites packed per stub tile (3*34 = 102 <= 128)
FINAL_SCALE = 1.0 / (float(B * C * H * W) * 4096.0)

_CACHE = {}


def _tile_geom(t):
    """(r0, R, off): output rows [r0, r0+R), partition p <-> image row off+p."""
    r0 = t * TILE_R
    R = TILE_R
    off = 0 if t == 0 else r0 - 2
    return r0, R, off


# --------------------------------------------------------------------------
# host-built stationary matrices (lhsT layout [k, m]: out[m] += M[k,m]*in[k])
# --------------------------------------------------------------------------

def _add_block(mats, pbase, off, r0, R, nrows):
    """Emit one vertical block's stencil/tri coefficients at partition base
    ``pbase``: block partitions m=0..nrows-1 <-> image rows off+m."""
    av, ihf, ihf1 = mats["AV"], mats["IHF"], mats["IHF1"]
    i71, s71, i72, s72, tri = (
        mats["I71"], mats["S71"], mats["I72"], mats["S72"], mats["TRI"])

    def sml_valid(m):
        r = off + m
        if not (max(r0 - 1, 0) <= r <= min(r0 + R, H - 1)):
            return False
        if m > nrows - 2 and r != H - 1:
            return False
        if m < 1 and r != 0:
            return False
        return True

    for m in range(nrows):
        if not sml_valid(m):
            continue
        r = off + m
        P = pbase + m
        ihf[P, P] += 1.0           # |c-lf|
        ihf1[P, P] += 1.0          # |c-rt|
        if r >= 1:
            av[P, P] += 1.0        # |c-up| = a_v[m]
        if r <= H - 2:
            av[P + 1, P] += 1.0    # |c-dn| = a_v[m+1]
        if r == 0:                 # up-row clamps to own row
            ihf[P, P] += DIAG_W    # |c-ul| -> a_h[m, f]
            ihf1[P, P] += DIAG_W   # |c-ur| -> a_h[m, f+1]
        else:
            i71[P, P] += DIAG_W    # |c-ul| = a_1[m]
            i72[P, P] += DIAG_W    # |c-ur| = a_2[m]
        if r == H - 1:             # down-row clamps to own row
            ihf1[P, P] += DIAG_W   # |c-dr| -> a_h[m, f+1]
            ihf[P, P] += DIAG_W    # |c-dl| -> a_h[m, f]
        else:
            s71[P + 1, P] += DIAG_W  # |c-dr| = a_1[m+1] at f+1
            s72[P + 1, P] += DIAG_W  # |c-dl| = a_2[m+1] at f-1

    for m in range(nrows):
        r = off + m
        if not (r0 <= r <= r0 + R - 1):
            continue
        for dr in (-1, 0, 1):
            k = m + dr
            if 0 <= k <= nrows - 1 and 0 <= off + k <= H - 1:
                tri[pbase + k, pbase + m] = 2.0 if dr == 0 else 1.0


def _matrices_for_unit(key):
    mats = {n: np.zeros((128, 128), np.float32)
            for n in ("AV", "IHF", "IHF1", "I71", "S71", "I72", "S72", "TRI")}
    if key == "stub":
        # 3 sites' bottom stubs packed at partition bases 0/34/68
        for b in range(STUB_SITES):
            _add_block(mats, b * STUB_NROWS, STUB_OFF, STUB_R0,
                       H - STUB_R0, STUB_NROWS)
    else:
        r0, R, off = _tile_geom(key)
        _add_block(mats, 0, off, r0, R, 128)
    return {k: v.astype(np.float16) for k, v in mats.items()}


def _build_weights():
    slots = {}
    packed = []
    index = {}
    for key in list(range(NFULL)) + ["stub"]:
        for name, mat in _matrices_for_unit(key).items():
            kb = mat.tobytes()
            if kb not in slots:
                slots[kb] = len(packed)
                packed.append(mat)
            index[(key, name)] = slots[kb]
    ones = np.zeros((128, 128), np.float16)
    ones[:, 0] = 1.0
    index[("ones",)] = len(packed)
    packed.append(ones)
    wts = np.concatenate(packed, axis=1)  # [128, NW*128]
    return np.ascontiguousarray(wts), index


# --------------------------------------------------------------------------
# custom DVE op: absdiff  out = |in0 - in1|, with a 2X_1PORT uop program
# --------------------------------------------------------------------------

def _register_absdiff():
    from concourse import dve_ops
    from concourse.dve_spec import Spec, Src0, Src1, maxx, lower
    from concourse.dve_uop import (
        DveOpSpec, UopConfig, UopDpConfig, AluOp, AluInp, DelayInp,
        InpSel, OutPath, OutSel,
    )
    from concourse.dve_ops import _COMPILE_CACHE

    NAME = "ABSDIFF_ANT"
    for op in dve_ops.OPS:
        if op.name == NAME:
            return op

    spec = Spec(
        body=maxx(Src0 - Src1, Src1 - Src0),
        reference=lambda in0, in1, s0, s1, imm2: np.abs(
            in0.astype(np.float32) - in1.astype(np.float32)
        ),
    )

    def build_2x(u1x):
        import copy

        u = copy.deepcopy(u1x)
        u.inp[3] = InpSel.SRC_0_HI
        u.inp[4] = InpSel.SRC_1_HI
        u.inp_enable[3] = 1
        u.inp_enable[4] = 1

        P = DelayInp.PREV_DELAY
        A = DelayInp.PREV_ALU_OUT

        def blk(op_, a, b, delay_sel, delay_en):
            d = UopDpConfig()
            d.op = op_
            d.alu_src0 = a
            d.alu_src1 = b
            d.alu_out_enable = 1
            d.delay = list(delay_sel) + [A] * (len(d.delay) - len(delay_sel))
            d.delay_enable = list(delay_en) + [0] * (
                len(d.delay_enable) - len(delay_en)
            )
            return d

        D0, D1, D2, D3 = (
            AluInp.PREV_DELAY_0, AluInp.PREV_DELAY_1,
            AluInp.PREV_DELAY_2, AluInp.PREV_DELAY_3,
        )
        ALU = AluInp.PREV_ALU_OUT
        u.datapath_config = [
            # lo: |s0-s1| on blks 0-2; hi (SRC_*_HI) on blks 3-5; the lo
            # result rides delay lane 0 to the output stage.
            blk(AluOp.SUBTRACT, D0, D1, [P, P, P, P], [1, 1, 1, 1]),
            blk(AluOp.SUBTRACT, D1, D0, [A, P, P, P], [1, 0, 1, 1]),
            blk(AluOp.MAX, D0, ALU, [P, P, P, P], [1, 0, 1, 1]),
            blk(AluOp.SUBTRACT, D2, D3, [A, P, P, P], [1, 0, 1, 1]),
            blk(AluOp.SUBTRACT, D3, D2, [P, A, P, P], [1, 1, 0, 0]),
            blk(AluOp.MAX, D1, ALU, [P, P, P, P], [1, 0, 0, 0]),
            blk(AluOp.BYPASS, ALU, ALU, [P, P, P, P], [1, 0, 0, 0]),
            blk(AluOp.BYPASS, ALU, ALU, [P, P, P, P], [1, 0, 0, 0]),
        ]
        u.out = dict(u.out)
        u.out[OutPath.WR0_LO] = OutSel.DELAY_0
        u.out[OutPath.WR0_HI] = OutSel.ALU_OUT
        u.out_enable = dict(u.out_enable)
        u.out_enable[OutPath.WR0_LO] = 1
        u.out_enable[OutPath.WR0_HI] = 1
        return u

    opcode = max(dve_ops._SUB_OPCODE_FOR_NAME.values()) + 1
    assert opcode < 0x20
    shas = {}
    specs_by_ver = {}
    for ver in ("v3", "v4"):
        uops = lower(spec, ver=ver)
        assert len(uops) == 1
        u2x = build_2x(uops[0])
        u2x.validate(ver)
        s = DveOpSpec(
            name=NAME, opcode=opcode, uops=uops, uops_2x=[u2x],
            perf_max=1, rd1_en=True,
        )
        shas[ver] = s.sha(ver)
        specs_by_ver[ver] = s

    op = dve_ops.DveOp(NAME, spec, subdim=False, uops_sha=shas)
    dve_ops.OPS.append(op)
    dve_ops._SUB_OPCODE_FOR_NAME[NAME] = opcode
    dve_ops.CUSTOM_DVE_SPECS[NAME] = spec
    # Seed the compile cache so both instruction emission and
    # dve_table_for_ops use THIS spec (with the 2x program).
    for ver, s in specs_by_ver.items():
        s.opcode = dve_ops.get_dve_sub_opcode(NAME)
        _COMPILE_CACHE[(NAME, ver)] = s
    return op


# --------------------------------------------------------------------------
# kernel build
# --------------------------------------------------------------------------

def _build(wts_np):
    import concourse.bass as bass
    import concourse.tile as tile
    from concourse import bacc, mybir

    F16 = mybir.dt.float16
    F32 = mybir.dt.float32
    AF = mybir.ActivationFunctionType
    OP = mybir.AluOpType

    absdiff = _register_absdiff()

    nc = bacc.Bacc()
    dA = nc.dram_tensor("TA", [NSITE, H, W + 2], F16, kind="ExternalInput")
    dB = nc.dram_tensor("TB", [NSITE, H, W + 2], F16, kind="ExternalInput")
    dW = nc.dram_tensor("WTS", list(wts_np.shape), F16, kind="ExternalInput")
    dO = nc.dram_tensor("OUT", [128, 64], F32, kind="ExternalOutput")

    custom_insts = []

    def absd(out_ap, in0_ap, in1_ap):
        bi = nc.vector._custom_dve(absdiff, out=out_ap, in0=in0_ap, in1=in1_ap)
        custom_insts.append(bi)
        return bi

    with tile.TileContext(nc) as tc, ExitStack() as ctx:
        persist = ctx.enter_context(tc.tile_pool(name="persist", bufs=1))
        xp = ctx.enter_context(tc.tile_pool(name="xp", bufs=6))
        fields = ctx.enter_context(tc.tile_pool(name="fields", bufs=2))
        tails = ctx.enter_context(tc.tile_pool(name="tails", bufs=3))
        ps_sml = ctx.enter_context(tc.tile_pool(name="ps_sml", bufs=1, space="PSUM"))
        ps_r = ctx.enter_context(tc.tile_pool(name="ps_r", bufs=2, space="PSUM"))

        wsb = persist.tile(list(wts_np.shape), F16)
        # PE p-state warmup tile is zeroed FIRST on the Pool queue: the
        # weights-DMA descriptor generation would otherwise hold the Pool
        # engine and delay the warmup start by over a microsecond
        warm = persist.tile([128, 512], F16)
        nc.gpsimd.memset(warm[:, :], 0.0)
        # weights go on the gpsimd queue: the sync queue must start
        # streaming x tiles immediately
        nc.gpsimd.dma_start(wsb[:], dW[:])

        # split accumulator: units 0..47 in acc_main (shipped as soon as
        # unit 47's accumulate lands), last two units in acc_tail so the
        # final DMA on the critical path is only 16 columns. Zeroed fully:
        # unused cols are summed by the host.
        acc_main = persist.tile([128, 48], F32)
        acc_tail = persist.tile([128, 16], F32)
        nc.vector.memset(acc_main[:], 0.0)
        nc.vector.memset(acc_tail[:], 0.0)

        def acc_col(idx):
            if idx < 48:
                return acc_main[:, idx:idx + 1]
            return acc_tail[:, idx - 48:idx - 47]

        # PE p-state warmup: junk matmuls on the zeroed tile burn the
        # slow-ramp window while the first input DMAs are in flight, so the
        # first real assembly runs at full clock.
        # borrow the smlA PSUM slot for warmup: using tag "r" would shift
        # the r-slot round-robin parity and make every TRI matmul collide
        # with the previous unit's not-yet-read accumulator
        warm_ps = ps_sml.tile([128, 1024], F32, tag="smlA")
        for _ in range(14):
            nc.tensor.matmul(warm_ps[:, 0:512], warm[:, 0:128], warm[:, :],
                             start=True, stop=True)

        # pre-initialize pool slots so partitions/cols never touched by DMA
        # or compute hold finite data, not virgin SBUF
        for _ in range(6):
            xu0 = xp.tile([128, 1026], F16, tag="xu")
            nc.gpsimd.memset(xu0[0:1, :], 0.0)
        for _ in range(2):
            for tg in ("a_h", "a_1", "a_2"):
                f0 = fields.tile([128, 1026], F16, tag=tg, name=tg)
                nc.gpsimd.memset(f0[:, 1024:1026], 0.0)

        def WT(t, name):
            i = _WIDX[(t, name)]
            return wsb[:, i * 128:(i + 1) * 128]

        def image_pipeline(dram, s, t, tag):
            x = xp.tile([128, 1026], F16, tag="x")
            if t == "stub":
                # pack 3 sites' bottom stubs: partitions 34b+m <-> site
                # 3s+b row 990+m (partitions 102..127 keep stale-but-finite
                # data; stub matrices have zero rows there)
                for b in range(STUB_SITES):
                    nc.sync.dma_start(
                        x[b * STUB_NROWS:(b + 1) * STUB_NROWS, :],
                        dram[STUB_SITES * s + b,
                             STUB_OFF:STUB_OFF + STUB_NROWS, :])
            else:
                r0, R, off = _tile_geom(t)
                nc.sync.dma_start(x[:, :], dram[s, off:off + 128, :])
            # xu[p] = image row off+p-1 (partition-shifted SBUF copy).
            # Issued from the scalar queue so the x->xu data dependency
            # doesn't stall the sync queue's x prefetch stream.
            xu = xp.tile([128, 1026], F16, tag="xu")
            nc.scalar.dma_start(xu[1:128, :], x[0:127, :])

            # |diff| fields via the fused absdiff op
            # col conventions (img col of sb col j):
            #   a_v: j   a_h: j    a_1: j    a_2: j-1
            a_v = fields.tile([128, 1024], F16, tag="a_v")
            absd(a_v[:, :], x[:, 1:1025], xu[:, 1:1025])
            a_h = fields.tile([128, 1026], F16, tag="a_h", name="a_h")
            absd(a_h[:, 0:1025], x[:, 1:1026], x[:, 0:1025])
            a_1 = fields.tile([128, 1026], F16, tag="a_1", name="a_1")
            absd(a_1[:, 0:1025], x[:, 1:1026], xu[:, 0:1025])
            a_2 = fields.tile([128, 1026], F16, tag="a_2", name="a_2")
            absd(a_2[:, 0:1025], x[:, 0:1025], xu[:, 1:1026])

            # sml assembly on PE (PSUM accumulate), 2 chunks of 512 cols
            sml = ps_sml.tile([128, 1024], F32, tag=f"sml{tag}")
            for c in range(2):
                F0 = c * 512
                o = sml[:, F0:F0 + 512]
                mm = nc.tensor.matmul
                mm(o, WT(t, "AV"), a_v[:, F0:F0 + 512], start=True, stop=False)
                mm(o, WT(t, "IHF"), a_h[:, F0:F0 + 512], start=False, stop=False)
                mm(o, WT(t, "IHF1"), a_h[:, F0 + 1:F0 + 513], start=False, stop=False)
                mm(o, WT(t, "I71"), a_1[:, F0:F0 + 512], start=False, stop=False)
                mm(o, WT(t, "S71"), a_1[:, F0 + 1:F0 + 513], start=False, stop=False)
                mm(o, WT(t, "I72"), a_2[:, F0 + 1:F0 + 513], start=False, stop=False)
                mm(o, WT(t, "S72"), a_2[:, F0:F0 + 512], start=False, stop=True)

            s2 = fields.tile([128, 1024], F16, tag=f"s2{tag}")
            nc.scalar.activation(s2[:], sml[:], AF.Square)
            return s2

        # --- tail stages, software-pipelined across iterations -----------
        def emit_q(p):
            # q = s2a - s2b with zeroed edge cols (Pool; DVE is tighter)
            q = tails.tile([128, 1026], F16, tag="q")
            nc.gpsimd.memset(q[:, 0:1], 0.0)
            nc.gpsimd.memset(q[:, 1025:1026], 0.0)
            nc.gpsimd.tensor_sub(q[:, 1:1025], p["s2a"][:], p["s2b"][:])
            p["q"] = q

        def emit_t1th(p):
            # horizontal [1,2,1] as two window-2 adds on DVE:
            # u[j] = q[j] + q[j+1]; th[c] = u[c] + u[c+1]
            q = p["q"]
            u = tails.tile([128, 1026], F16, tag="t1")
            nc.vector.tensor_add(u[:, 0:1025], q[:, 0:1025], q[:, 1:1026])
            th = tails.tile([128, 1024], F16, tag="th")
            nc.gpsimd.tensor_add(th[:], u[:, 0:1024], u[:, 1:1025])
            p["th"] = th

        def emit_trijunk(p):
            th, t, idx = p["th"], p["t"], p["idx"]
            r = ps_r.tile([128, 1024], F32, tag="r")
            for c in range(2):
                nc.tensor.matmul(
                    r[:, c * 512:(c + 1) * 512], WT(t, "TRI"),
                    th[:, c * 512:(c + 1) * 512], start=True, stop=True,
                )
            junk = tails.tile([128, 1024], F32, tag="junk")
            nc.scalar.activation(junk[:], r[:], AF.Square,
                                 accum_out=acc_col(idx))

        # 3-stage pipeline: pipes(k) | q(k-1) | t1/th(k-1) | TRI/junk(k-2)
        # so PE's TRI and Pool's th always have a full iteration of slack.
        items = []
        units = [(s, t) for s in range(NSITE) for t in range(NFULL)]
        units += [(g, "stub") for g in range(NSITE // STUB_SITES)]
        for k, (s, t) in enumerate(units):
            s2a = image_pipeline(dA, s, t, "A")
            s2b = image_pipeline(dB, s, t, "B")
            items.append({"s2a": s2a, "s2b": s2b, "t": t, "idx": k})
            if k >= 1 and k - 1 < len(units) - 2:
                emit_q(items[k - 1])
                emit_t1th(items[k - 1])
            if k >= 2:
                emit_trijunk(items[k - 2])
        # drain fast-path: run the final two tails' horizontal stage on DVE
        # (short chain) so the kernel doesn't trail off on the slow Pool ops
        def fast_tail(p):
            q = tails.tile([128, 1026], F16, tag="q")
            nc.vector.memset(q[:, 0:1], 0.0)
            nc.vector.memset(q[:, 1025:1026], 0.0)
            nc.vector.tensor_sub(q[:, 1:1025], p["s2a"][:], p["s2b"][:])
            u = tails.tile([128, 1026], F16, tag="t1")
            nc.vector.tensor_add(u[:, 0:1025], q[:, 0:1025], q[:, 1:1026])
            th = tails.tile([128, 1024], F16, tag="th")
            nc.vector.tensor_add(th[:], u[:, 0:1024], u[:, 1:1025])
            p["th"] = th

        last = len(units) - 1
        fast_tail(items[last - 1])
        fast_tail(items[last])
        emit_trijunk(items[last - 1])
        emit_trijunk(items[last])

        # ship the partial accumulators; the host sums them together with
        # the cross-core reduction
        nc.sync.dma_start(dO[:, 0:48], acc_main[:])
        nc.sync.dma_start(dO[:, 48:64], acc_tail[:])

    for bi in custom_insts:
        bi.ins.perf_max = 1
    nc.compile()
    return nc


_WIDX = None


def _get_module():
    global _WIDX
    if "nc" in _CACHE:
        return _CACHE["nc"], _CACHE["wts"]
    wts_np, widx = _build_weights()
    _WIDX = widx
    nc = _build(wts_np)
    _CACHE["nc"] = nc
    _CACHE["wts"] = wts_np
    return nc, wts_np


def _pad_cols(a):
    # [NSITE, H, W] -> [NSITE, H, W+2] fp16 with edge-replicated columns
    out = np.empty((a.shape[0], a.shape[1], a.shape[2] + 2), np.float16)
    out[:, :, 1:-1] = a
    out[:, :, 0] = a[:, :, 0]
    out[:, :, -1] = a[:, :, -1]
    return out


def kernel(TensorA, TensorB):
    from concourse.bass_utils import run_bass_kernel_spmd

    nc, wts_np = _get_module()
    A = np.asarray(TensorA, dtype=np.float32).reshape(B * C, H, W)
    Bv = np.asarray(TensorB, dtype=np.float32).reshape(B * C, H, W)
    in_maps = []
    for c in range(NCORES):
        in_maps.append({
            "TA": _pad_cols(A[c * NSITE:(c + 1) * NSITE]),
            "TB": _pad_cols(Bv[c * NSITE:(c + 1) * NSITE]),
            "WTS": wts_np,
        })
    res = run_bass_kernel_spmd(
        nc, in_maps, core_ids=list(range(NCORES)),
        trace=bool(int(os.environ.get("CLAR_TRACE", "0"))),
    )
    _CACHE["last_results"] = res
    total = sum(float(r["OUT"].astype(np.float64).sum()) for r in res.results)
    return np.float32(total * FINAL_SCALE)


# revision 53
# speedup vs baseline: 1.5936x; 1.0053x over previous
"""Trainium2 Bass kernel for nn_Clar_Loss (NSML fusion-clarity MSE loss).

reference:
    x = (t+1)/2 ; s = sml(x) (8-neighbor abs-diff stencil, edge-replicate)
    nsml = G (*) s^2  (3x3 gaussian [[1,2,1],[2,4,2],[1,2,1]]/16, zero pad)
    loss = mean((nsml(A) - nsml(B))^2)

Algebra used here:
    sml((t+1)/2) = sml(t)/2          (translation invariant, pos. homogeneous)
    loss = sum((Graw (*) q)^2) / (N * 4096),  q = sA^2 - sB^2 (raw sml)
with Graw = [[1,2,1],[2,4,2],[1,2,1]] (integer), separable [1,2,1](x)[1,2,1].

Sharding: pure data-parallel over the batch dim (16 -> 2 per core); each core
returns a partial sum of (Graw(*)q)^2; host sums and rescales.

Layout: inputs are pre-padded on host to [H, W+2] (edge-replicated columns)
and pre-cast to fp16, so every row-tile is ONE full [128, 1026] fp16 DMA
issued from the SP queue (no gpsimd cast-DMA needed; xu shift-copies go on
the scalar queue so their x->xu data dependency never stalls the x prefetch
stream). Partition p <-> image row off+p with 8 full 124-row tiles per
image; the three sites' bottom 32-row stubs are PACKED into one extra tile
(partition bases 0/34/68), so each core runs 50 uniform pair-tile units
instead of 54. Vertical/partition shifts are folded into host-built 128x128
fp16 stationary matrices; image-boundary clamping and the zero padding of
s^2 are encoded as matrix edits, so no partition ever needs pad data.

All four |diff| fields are produced by a custom fused DVE absdiff op
(max(a-b, b-a)) registered with a hand-written 2X_1PORT uop program and
perf_max=1, so fp16 packed operands run at 2 elem/lane/cycle. Per pair-tile
the engines run: PE 28 assembly + 2 TRI matmuls (the saturated bottleneck,
kept gap-free so the p-state stays at full clock — warmup matmuls burn the
slow-ramp window during the first DMAs); DVE 8 absdiffs + the u window-add;
Pool q + th + edge memsets; ACT the two squares + the square-accumulate.
Tails are software-pipelined behind the next unit's field production, with
TRI/accumulate delayed a second iteration so PE never waits. Per-unit
partial sums land in a [128, 64] SBUF accumulator DMA'd out whole; the host
does the final sum and cross-core reduction.
"""

import os
from contextlib import ExitStack

import numpy as np

B, C, H, W = 16, 3, 1024, 1024
NCORES = 8
BPC = B // NCORES            # batch per core
NSITE = BPC * C              # image pairs per core
DIAG_W = 0.707
TILE_R = 124
NFULL = 8                    # full tiles 0..7 cover output rows 0..991
STUB_OFF = 990               # stub block: x rows 990..1023
STUB_R0 = 992                # stub outputs rows 992..1023 (32 rows)
STUB_NROWS = 34
STUB_SITES = 3               # sites packed per stub tile (3*34 = 102 <= 128)
FINAL_SCALE = 1.0 / (float(B * C * H * W) * 4096.0)

_CACHE = {}


def _tile_geom(t):
    """(r0, R, off): output rows [r0, r0+R), partition p <-> image row off+p."""
    r0 = t * TILE_R
    R = TILE_R
    off = 0 if t == 0 else r0 - 2
    return r0, R, off


# --------------------------------------------------------------------------
# host-built stationary matrices (lhsT layout [k, m]: out[m] += M[k,m]*in[k])
# --------------------------------------------------------------------------

def _add_block(mats, pbase, off, r0, R, nrows):
    """Emit one vertical block's stencil/tri coefficients at partition base
    ``pbase``: block partitions m=0..nrows-1 <-> image rows off+m."""
    av, ihf, ihf1 = mats["AV"], mats["IHF"], mats["IHF1"]
    i71, s71, i72, s72, tri = (
        mats["I71"], mats["S71"], mats["I72"], mats["S72"], mats["TRI"])

    def sml_valid(m):
        r = off + m
        if not (max(r0 - 1, 0) <= r <= min(r0 + R, H - 1)):
            return False
        if m > nrows - 2 and r != H - 1:
            return False
        if m < 1 and r != 0:
            return False
        return True

    for m in range(nrows):
        if not sml_valid(m):
            continue
        r = off + m
        P = pbase + m
        ihf[P, P] += 1.0           # |c-lf|
        ihf1[P, P] += 1.0          # |c-rt|
        if r >= 1:
            av[P, P] += 1.0        # |c-up| = a_v[m]
        if r <= H - 2:
            av[P + 1, P] += 1.0    # |c-dn| = a_v[m+1]
        if r == 0:                 # up-row clamps to own row
            ihf[P, P] += DIAG_W    # |c-ul| -> a_h[m, f]
            ihf1[P, P] += DIAG_W   # |c-ur| -> a_h[m, f+1]
        else:
            i71[P, P] += DIAG_W    # |c-ul| = a_1[m]
            i72[P, P] += DIAG_W    # |c-ur| = a_2[m]
        if r == H - 1:             # down-row clamps to own row
            ihf1[P, P] += DIAG_W   # |c-dr| -> a_h[m, f+1]
            ihf[P, P] += DIAG_W    # |c-dl| -> a_h[m, f]
        else:
            s71[P + 1, P] += DIAG_W  # |c-dr| = a_1[m+1] at f+1
            s72[P + 1, P] += DIAG_W  # |c-dl| = a_2[m+1] at f-1

    for m in range(nrows):
        r = off + m
        if not (r0 <= r <= r0 + R - 1):
            continue
        for dr in (-1, 0, 1):
            k = m + dr
            if 0 <= k <= nrows - 1 and 0 <= off + k <= H - 1:
                tri[pbase + k, pbase + m] = 2.0 if dr == 0 else 1.0


def _matrices_for_unit(key):
    mats = {n: np.zeros((128, 128), np.float32)
            for n in ("AV", "IHF", "IHF1", "I71", "S71", "I72", "S72", "TRI")}
    if key == "stub":
        # 3 sites' bottom stubs packed at partition bases 0/34/68
        for b in range(STUB_SITES):
            _add_block(mats, b * STUB_NROWS, STUB_OFF, STUB_R0,
                       H - STUB_R0, STUB_NROWS)
    else:
        r0, R, off = _tile_geom(key)
        _add_block(mats, 0, off, r0, R, 128)
    return {k: v.astype(np.float16) for k, v in mats.items()}


def _build_weights():
    slots = {}
    packed = []
    index = {}
    n_hot = 0
    for key in list(range(NFULL)) + ["stub"]:
        for name, mat in _matrices_for_unit(key).items():
            kb = mat.tobytes()
            if kb not in slots:
                slots[kb] = len(packed)
                packed.append(mat)
            index[(key, name)] = slots[kb]
        if key == NFULL - 1:
            # slots used by the full tiles (units 0..47); the stub slots
            # after this are only needed by the last two units, so their
            # DMA can trail the startup-critical transfers
            n_hot = len(packed)
    index["n_hot"] = n_hot
    wts = np.concatenate(packed, axis=1)  # [128, NW*128]
    return np.ascontiguousarray(wts), index


# --------------------------------------------------------------------------
# custom DVE op: absdiff  out = |in0 - in1|, with a 2X_1PORT uop program
# --------------------------------------------------------------------------

def _register_absdiff():
    from concourse import dve_ops
    from concourse.dve_spec import Spec, Src0, Src1, maxx, lower
    from concourse.dve_uop import (
        DveOpSpec, UopConfig, UopDpConfig, AluOp, AluInp, DelayInp,
        InpSel, OutPath, OutSel,
    )
    from concourse.dve_ops import _COMPILE_CACHE

    NAME = "ABSDIFF_ANT"
    for op in dve_ops.OPS:
        if op.name == NAME:
            return op

    spec = Spec(
        body=maxx(Src0 - Src1, Src1 - Src0),
        reference=lambda in0, in1, s0, s1, imm2: np.abs(
            in0.astype(np.float32) - in1.astype(np.float32)
        ),
    )

    def build_2x(u1x):
        import copy

        u = copy.deepcopy(u1x)
        u.inp[3] = InpSel.SRC_0_HI
        u.inp[4] = InpSel.SRC_1_HI
        u.inp_enable[3] = 1
        u.inp_enable[4] = 1

        P = DelayInp.PREV_DELAY
        A = DelayInp.PREV_ALU_OUT

        def blk(op_, a, b, delay_sel, delay_en):
            d = UopDpConfig()
            d.op = op_
            d.alu_src0 = a
            d.alu_src1 = b
            d.alu_out_enable = 1
            d.delay = list(delay_sel) + [A] * (len(d.delay) - len(delay_sel))
            d.delay_enable = list(delay_en) + [0] * (
                len(d.delay_enable) - len(delay_en)
            )
            return d

        D0, D1, D2, D3 = (
            AluInp.PREV_DELAY_0, AluInp.PREV_DELAY_1,
            AluInp.PREV_DELAY_2, AluInp.PREV_DELAY_3,
        )
        ALU = AluInp.PREV_ALU_OUT
        u.datapath_config = [
            # lo: |s0-s1| on blks 0-2; hi (SRC_*_HI) on blks 3-5; the lo
            # result rides delay lane 0 to the output stage.
            blk(AluOp.SUBTRACT, D0, D1, [P, P, P, P], [1, 1, 1, 1]),
            blk(AluOp.SUBTRACT, D1, D0, [A, P, P, P], [1, 0, 1, 1]),
            blk(AluOp.MAX, D0, ALU, [P, P, P, P], [1, 0, 1, 1]),
            blk(AluOp.SUBTRACT, D2, D3, [A, P, P, P], [1, 0, 1, 1]),
            blk(AluOp.SUBTRACT, D3, D2, [P, A, P, P], [1, 1, 0, 0]),
            blk(AluOp.MAX, D1, ALU, [P, P, P, P], [1, 0, 0, 0]),
            blk(AluOp.BYPASS, ALU, ALU, [P, P, P, P], [1, 0, 0, 0]),
            blk(AluOp.BYPASS, ALU, ALU, [P, P, P, P], [1, 0, 0, 0]),
        ]
        u.out = dict(u.out)
        u.out[OutPath.WR0_LO] = OutSel.DELAY_0
        u.out[OutPath.WR0_HI] = OutSel.ALU_OUT
        u.out_enable = dict(u.out_enable)
        u.out_enable[OutPath.WR0_LO] = 1
        u.out_enable[OutPath.WR0_HI] = 1
        return u

    opcode = max(dve_ops._SUB_OPCODE_FOR_NAME.values()) + 1
    assert opcode < 0x20
    shas = {}
    specs_by_ver = {}
    for ver in ("v3", "v4"):
        uops = lower(spec, ver=ver)
        assert len(uops) == 1
        u2x = build_2x(uops[0])
        u2x.validate(ver)
        s = DveOpSpec(
            name=NAME, opcode=opcode, uops=uops, uops_2x=[u2x],
            perf_max=1, rd1_en=True,
        )
        shas[ver] = s.sha(ver)
        specs_by_ver[ver] = s

    op = dve_ops.DveOp(NAME, spec, subdim=False, uops_sha=shas)
    dve_ops.OPS.append(op)
    dve_ops._SUB_OPCODE_FOR_NAME[NAME] = opcode
    dve_ops.CUSTOM_DVE_SPECS[NAME] = spec
    # Seed the compile cache so both instruction emission and
    # dve_table_for_ops use THIS spec (with the 2x program).
    for ver, s in specs_by_ver.items():
        s.opcode = dve_ops.get_dve_sub_opcode(NAME)
        _COMPILE_CACHE[(NAME, ver)] = s
    return op


# --------------------------------------------------------------------------
# kernel build
# --------------------------------------------------------------------------

def _build(wts_np):
    import concourse.bass as bass
    import concourse.tile as tile
    from concourse import bacc, mybir

    F16 = mybir.dt.float16
    F32 = mybir.dt.float32
    AF = mybir.ActivationFunctionType
    OP = mybir.AluOpType

    absdiff = _register_absdiff()

    nc = bacc.Bacc()
    dA = nc.dram_tensor("TA", [NSITE, H, W + 2], F16, kind="ExternalInput")
    dB = nc.dram_tensor("TB", [NSITE, H, W + 2], F16, kind="ExternalInput")
    dW = nc.dram_tensor("WTS", list(wts_np.shape), F16, kind="ExternalInput")
    dO = nc.dram_tensor("OUT", [128, 64], F32, kind="ExternalOutput")

    custom_insts = []

    def absd(out_ap, in0_ap, in1_ap):
        bi = nc.vector._custom_dve(absdiff, out=out_ap, in0=in0_ap, in1=in1_ap)
        custom_insts.append(bi)
        return bi

    with tile.TileContext(nc) as tc, ExitStack() as ctx:
        persist = ctx.enter_context(tc.tile_pool(name="persist", bufs=1))
        xp = ctx.enter_context(tc.tile_pool(name="xp", bufs=4))
        fields = ctx.enter_context(tc.tile_pool(name="fields", bufs=2))
        tails = ctx.enter_context(tc.tile_pool(name="tails", bufs=3))
        ps_sml = ctx.enter_context(tc.tile_pool(name="ps_sml", bufs=1, space="PSUM"))
        ps_r = ctx.enter_context(tc.tile_pool(name="ps_r", bufs=2, space="PSUM"))

        # PE p-state warmup tile is zeroed FIRST on the Pool queue: the
        # weights-DMA descriptor generation would otherwise hold the Pool
        # engine and delay the warmup start by over a microsecond
        warm = persist.tile([128, 512], F16)
        nc.gpsimd.memset(warm[:, :], 0.0)
        # weights split hot/cold on the gpsimd queue: the hot (full-tile)
        # matrices load first so the startup-critical xu copy isn't stuck
        # behind the whole table on the shared DMA engines; the stub
        # matrices trail (first needed ~300us in)
        n_hot = _WIDX["n_hot"]
        ca = n_hot * 128
        wsb_a = persist.tile([128, ca], F16)
        wsb_b = persist.tile([128, wts_np.shape[1] - ca], F16)
        nc.gpsimd.dma_start(wsb_a[:], dW[:, 0:ca])

        # split accumulator: units 0..47 in acc_main (shipped as soon as
        # unit 47's accumulate lands), last two units in acc_tail so the
        # final DMA on the critical path is only 16 columns. Zeroed fully:
        # unused cols are summed by the host.
        acc_main = persist.tile([128, 48], F32)
        acc_tail = persist.tile([128, 16], F32)
        nc.vector.memset(acc_main[:], 0.0)
        nc.vector.memset(acc_tail[:], 0.0)

        def acc_col(idx):
            if idx < 48:
                return acc_main[:, idx:idx + 1]
            return acc_tail[:, idx - 48:idx - 47]

        # PE p-state warmup: junk matmuls on the zeroed tile burn the
        # slow-ramp window while the first input DMAs are in flight, so the
        # first real assembly runs at full clock.
        # borrow the smlA PSUM slot for warmup: using tag "r" would shift
        # the r-slot round-robin parity and make every TRI matmul collide
        # with the previous unit's not-yet-read accumulator
        warm_ps = ps_sml.tile([128, 1024], F32, tag="smlA")
        for _ in range(14):
            nc.tensor.matmul(warm_ps[:, 0:512], warm[:, 0:128], warm[:, :],
                             start=True, stop=True)

        # pre-initialize pool slots so partitions/cols never touched by DMA
        # or compute hold finite data, not virgin SBUF
        for _ in range(4):
            xu0 = xp.tile([128, 1026], F16, tag="xu")
            nc.gpsimd.memset(xu0[0:1, :], 0.0)
        for _ in range(2):
            for tg in ("a_h", "a_1", "a_2"):
                f0 = fields.tile([128, 1026], F16, tag=tg, name=tg)
                nc.gpsimd.memset(f0[:, 1024:1026], 0.0)

        def WT(t, name):
            i = _WIDX[(t, name)]
            if i < n_hot:
                return wsb_a[:, i * 128:(i + 1) * 128]
            j = i - n_hot
            return wsb_b[:, j * 128:(j + 1) * 128]

        def image_pipeline(dram, s, t, tag):
            x = xp.tile([128, 1026], F16, tag="x")
            if t == "stub":
                # pack 3 sites' bottom stubs: partitions 34b+m <-> site
                # 3s+b row 990+m (partitions 102..127 keep stale-but-finite
                # data; stub matrices have zero rows there)
                for b in range(STUB_SITES):
                    nc.sync.dma_start(
                        x[b * STUB_NROWS:(b + 1) * STUB_NROWS, :],
                        dram[STUB_SITES * s + b,
                             STUB_OFF:STUB_OFF + STUB_NROWS, :])
            else:
                r0, R, off = _tile_geom(t)
                nc.sync.dma_start(x[:, :], dram[s, off:off + 128, :])
            # xu[p] = image row off+p-1 (partition-shifted SBUF copy).
            # Issued from the scalar queue so the x->xu data dependency
            # doesn't stall the sync queue's x prefetch stream.
            xu = xp.tile([128, 1026], F16, tag="xu")
            nc.scalar.dma_start(xu[1:128, :], x[0:127, :])

            # |diff| fields via the fused absdiff op
            # col conventions (img col of sb col j):
            #   a_v: j   a_h: j    a_1: j    a_2: j-1
            a_v = fields.tile([128, 1024], F16, tag="a_v")
            absd(a_v[:, :], x[:, 1:1025], xu[:, 1:1025])
            a_h = fields.tile([128, 1026], F16, tag="a_h", name="a_h")
            absd(a_h[:, 0:1025], x[:, 1:1026], x[:, 0:1025])
            a_1 = fields.tile([128, 1026], F16, tag="a_1", name="a_1")
            absd(a_1[:, 0:1025], x[:, 1:1026], xu[:, 0:1025])
            a_2 = fields.tile([128, 1026], F16, tag="a_2", name="a_2")
            absd(a_2[:, 0:1025], x[:, 0:1025], xu[:, 1:1026])

            # sml assembly on PE (PSUM accumulate), 2 chunks of 512 cols
            sml = ps_sml.tile([128, 1024], F32, tag=f"sml{tag}")
            for c in range(2):
                F0 = c * 512
                o = sml[:, F0:F0 + 512]
                mm = nc.tensor.matmul
                mm(o, WT(t, "AV"), a_v[:, F0:F0 + 512], start=True, stop=False)
                mm(o, WT(t, "IHF"), a_h[:, F0:F0 + 512], start=False, stop=False)
                mm(o, WT(t, "IHF1"), a_h[:, F0 + 1:F0 + 513], start=False, stop=False)
                mm(o, WT(t, "I71"), a_1[:, F0:F0 + 512], start=False, stop=False)
                mm(o, WT(t, "S71"), a_1[:, F0 + 1:F0 + 513], start=False, stop=False)
                mm(o, WT(t, "I72"), a_2[:, F0 + 1:F0 + 513], start=False, stop=False)
                mm(o, WT(t, "S72"), a_2[:, F0:F0 + 512], start=False, stop=True)

            s2 = fields.tile([128, 1024], F16, tag=f"s2{tag}")
            nc.scalar.activation(s2[:], sml[:], AF.Square)
            return s2

        # --- tail stages, software-pipelined across iterations -----------
        def emit_q(p):
            # q = s2a - s2b with zeroed edge cols (Pool; DVE is tighter)
            q = tails.tile([128, 1026], F16, tag="q")
            nc.gpsimd.memset(q[:, 0:1], 0.0)
            nc.gpsimd.memset(q[:, 1025:1026], 0.0)
            nc.gpsimd.tensor_sub(q[:, 1:1025], p["s2a"][:], p["s2b"][:])
            p["q"] = q

        def emit_t1th(p):
            # horizontal [1,2,1] as two window-2 adds on DVE:
            # u[j] = q[j] + q[j+1]; th[c] = u[c] + u[c+1]
            q = p["q"]
            u = tails.tile([128, 1026], F16, tag="t1")
            nc.vector.tensor_add(u[:, 0:1025], q[:, 0:1025], q[:, 1:1026])
            th = tails.tile([128, 1024], F16, tag="th")
            nc.gpsimd.tensor_add(th[:], u[:, 0:1024], u[:, 1:1025])
            p["th"] = th

        def emit_trijunk(p):
            th, t, idx = p["th"], p["t"], p["idx"]
            r = ps_r.tile([128, 1024], F32, tag="r")
            for c in range(2):
                nc.tensor.matmul(
                    r[:, c * 512:(c + 1) * 512], WT(t, "TRI"),
                    th[:, c * 512:(c + 1) * 512], start=True, stop=True,
                )
            junk = tails.tile([128, 1024], F32, tag="junk")
            nc.scalar.activation(junk[:], r[:], AF.Square,
                                 accum_out=acc_col(idx))

        # 3-stage pipeline: pipes(k) | q(k-1) | t1/th(k-1) | TRI/junk(k-2)
        # so PE's TRI and Pool's th always have a full iteration of slack.
        items = []
        units = [(s, t) for s in range(NSITE) for t in range(NFULL)]
        units += [(g, "stub") for g in range(NSITE // STUB_SITES)]
        for k, (s, t) in enumerate(units):
            s2a = image_pipeline(dA, s, t, "A")
            s2b = image_pipeline(dB, s, t, "B")
            items.append({"s2a": s2a, "s2b": s2b, "t": t, "idx": k})
            if k == 1:
                # trail the stub-matrix load behind the first unit's traffic
                nc.gpsimd.dma_start(wsb_b[:], dW[:, ca:wts_np.shape[1]])
            if k >= 1 and k - 1 < len(units) - 2:
                emit_q(items[k - 1])
                emit_t1th(items[k - 1])
            if k >= 2:
                emit_trijunk(items[k - 2])
        # drain fast-path: run the final two tails' horizontal stage on DVE
        # (short chain) so the kernel doesn't trail off on the slow Pool ops
        def fast_tail(p):
            q = tails.tile([128, 1026], F16, tag="q")
            nc.vector.memset(q[:, 0:1], 0.0)
            nc.vector.memset(q[:, 1025:1026], 0.0)
            nc.vector.tensor_sub(q[:, 1:1025], p["s2a"][:], p["s2b"][:])
            u = tails.tile([128, 1026], F16, tag="t1")
            nc.vector.tensor_add(u[:, 0:1025], q[:, 0:1025], q[:, 1:1026])
            th = tails.tile([128, 1024], F16, tag="th")
            nc.vector.tensor_add(th[:], u[:, 0:1024], u[:, 1:1025])
            p["th"] = th

        last = len(units) - 1
        fast_tail(items[last - 1])
        fast_tail(items[last])
        emit_trijunk(items[last - 1])
        emit_trijunk(items[last])

        # ship the partial accumulators; the host sums them together with
        # the cross-core reduction
        nc.sync.dma_start(dO[:, 0:48], acc_main[:])
        nc.sync.dma_start(dO[:, 48:64], acc_tail[:])

    for bi in custom_insts:
        bi.ins.perf_max = 1
    nc.compile()
    return nc


_WIDX = None


def _get_module():
    global _WIDX
    if "nc" in _CACHE:
        return _CACHE["nc"], _CACHE["wts"]
    wts_np, widx = _build_weights()
    _WIDX = widx
    nc = _build(wts_np)
    _CACHE["nc"] = nc
    _CACHE["wts"] = wts_np
    return nc, wts_np


def _pad_cols(a):
    # [NSITE, H, W] -> [NSITE, H, W+2] fp16 with edge-replicated columns
    out = np.empty((a.shape[0], a.shape[1], a.shape[2] + 2), np.float16)
    out[:, :, 1:-1] = a
    out[:, :, 0] = a[:, :, 0]
    out[:, :, -1] = a[:, :, -1]
    return out


def kernel(TensorA, TensorB):
    from concourse.bass_utils import run_bass_kernel_spmd

    nc, wts_np = _get_module()
    A = np.asarray(TensorA, dtype=np.float32).reshape(B * C, H, W)
    Bv = np.asarray(TensorB, dtype=np.float32).reshape(B * C, H, W)
    in_maps = []
    for c in range(NCORES):
        in_maps.append({
            "TA": _pad_cols(A[c * NSITE:(c + 1) * NSITE]),
            "TB": _pad_cols(Bv[c * NSITE:(c + 1) * NSITE]),
            "WTS": wts_np,
        })
    res = run_bass_kernel_spmd(
        nc, in_maps, core_ids=list(range(NCORES)),
        trace=bool(int(os.environ.get("CLAR_TRACE", "0"))),
    )
    _CACHE["last_results"] = res
    total = sum(float(r["OUT"].astype(np.float64).sum()) for r in res.results)
    return np.float32(total * FINAL_SCALE)
